# revision 7
# baseline (speedup 1.0000x reference)
"""Bass/Trainium2 kernel for nn_DiscriminativeCorrelationFilter.

Math
----
Reference computes, per batch b:
  sp = BN(W @ xs_b), tp = BN(W @ xt_b)        (1x1 conv 768->768 + eval-mode BN)
  label from mask centroid (Gaussian)
  f_0 = f_init;  5 iterations:
      r = f_t . tp  (per pixel);  cond = (r*label < 1)
      grad_b = mean(cond * (-label*mask))     (a SCALAR per batch)
      f_{t+1} = (1-LR*LAM) f_t - LR*grad_b*ones
  out_b = f_5 . sp

Because BN(W@x) = inv_std .* (W@x) + cvec (affine per channel) and f_t
stays in span{f_init, ones} (the gradient is a per-batch scalar):
  f_t = a_t * f_init + c_t * ones,  a_t = rho^t  (compile-time)
every channel contraction collapses onto two fixed vectors
    p = W^T (f_init .* inv_std),  q = W^T inv_std          (768 each)
with scalars k1 = f_init.cvec, k2 = sum(cvec):
    f_t . BN(W@x) = a_t (p^T x + k1) + c_t (q^T x + k2)
Device work per batch:
  u = p^T xt + k1, s = q^T xt + k2                      (256 each)
  recurrence on ctil_t = c_t/a_t:
    resp = u*lab + ctil_t * (s*lab); cond = resp < rho^-t
    gsum = sum(cond * glm), glm = (LR/256)*label*mask
    ctil_{t+1} = ctil_t + gsum * rho^-(t+1)
  out = a5 * (G^T xs) + (a5 k1 + a5 k2 ctil5),  G = p + ctil5 * q
p, q, k1, k2, label, glm are cheap host precomputes from the small
weights; the 126 MB of features stream through skinny matmuls once,
so the kernel is DMA-bound (~15.7 MB/core).

Sharding: data-parallel over batch, 4 batches per core on 8 cores.
Engine ops keep all SBUF operands at partition base 0 (HW requires
base in {0,32,64,96}); partition rearrangements go through small
SBUF->SBUF DMAs, which have no base restriction.
"""

import numpy as np
from contextlib import ExitStack

import concourse.bass as bass
import concourse.bacc as bacc
import concourse.mybir as mybir
import concourse.tile as tile
from concourse.bass_utils import run_bass_kernel_spmd

# ---------------- problem constants (hardcoded; kernel.py must be standalone)
B = 32            # full batch
D = 768           # feature dim
HS = WS = 32      # search spatial
HT = WT = 16      # target spatial
NS = HS * WS      # 1024
NT = HT * WT      # 256
NCORES = 8
BPC = B // NCORES  # 4 batches per core
KC = D // 128      # 6 contraction chunks

LR = 0.1
LAM = 0.01
SIGMA = 2.0
NIT = 5
BN_EPS = 1e-5
RHO = 1.0 - LR * LAM          # 0.999
A5 = RHO ** NIT

F32 = mybir.dt.float32
F32R = mybir.dt.float32r

USE_F32R = False   # stream feature matmuls as float32r (full-rate fp32 on PE)

_CACHE = {}


def _mm(ap):
    return ap.bitcast(F32R) if USE_F32R else ap


def build():
    """Build the per-core Bass program (shapes only; no input values baked)."""
    nc = bacc.Bacc()
    xt = nc.dram_tensor("xt", (BPC, D, NT), F32, kind="ExternalInput")
    xs = nc.dram_tensor("xs", (BPC, D, NS), F32, kind="ExternalInput")
    pq = nc.dram_tensor("pq", (D, 2), F32, kind="ExternalInput")
    lab = nc.dram_tensor("lab", (BPC, NT), F32, kind="ExternalInput")
    glm = nc.dram_tensor("glm", (BPC, NT), F32, kind="ExternalInput")
    karr = nc.dram_tensor("karr", (BPC, 4), F32, kind="ExternalInput")
    out = nc.dram_tensor("out", (BPC, NS), F32, kind="ExternalOutput")

    AL = mybir.AluOpType
    AF = mybir.ActivationFunctionType

    with tile.TileContext(nc) as tc, ExitStack() as ctx:
        const = ctx.enter_context(tc.tile_pool(name="const", bufs=1))
        feats = ctx.enter_context(tc.tile_pool(name="feats", bufs=1))
        work = ctx.enter_context(tc.tile_pool(name="work", bufs=1))
        psum = ctx.enter_context(tc.tile_pool(name="psum", bufs=8, space="PSUM"))

        # ---- small constant loads
        pq_sb = const.tile([128, KC, 2], F32, tag="pq")
        nc.sync.dma_start(pq_sb[:, :, :], pq.rearrange("(k p) c -> p k c", p=128))
        lab_sb = const.tile([BPC, NT], F32, tag="lab")
        nc.sync.dma_start(lab_sb[:, :], lab[:, :])
        glm_sb = const.tile([BPC, NT], F32, tag="glm")
        nc.sync.dma_start(glm_sb[:, :], glm[:, :])
        karr_sb = const.tile([BPC, 4], F32, tag="karr")
        nc.sync.dma_start(karr_sb[:, :], karr[:, :])

        # ---- feature loads (target first: it gates the serial recurrence)
        xt_sb = []
        for k in range(KC):
            t = feats.tile([128, BPC, NT], F32, tag=f"xt{k}", name=f"xt{k}")
            nc.sync.dma_start(
                t[:, :, :], xt[:, k * 128:(k + 1) * 128, :].rearrange("b p n -> p b n")
            )
            xt_sb.append(t)
        xs_sb = []
        for k in range(KC):
            t = feats.tile([128, BPC, NS], F32, tag=f"xs{k}", name=f"xs{k}")
            nc.sync.dma_start(
                t[:, :, :], xs[:, k * 128:(k + 1) * 128, :].rearrange("b p n -> p b n")
            )
            xs_sb.append(t)

        # ---- target stage: psT[j] (2,512) = [p;q]^T @ xt for batches (2j, 2j+1)
        psT = [psum.tile([2, 512], F32, tag="ps", name=f"psT{j}") for j in range(2)]
        for j in range(2):
            for k in range(KC):
                nc.tensor.matmul(
                    psT[j][:, :],
                    _mm(pq_sb[:, k, :]),
                    _mm(xt_sb[k][:, 2 * j:2 * j + 2, :]),
                    start=(k == 0),
                    stop=(k == KC - 1),
                )

        # ---- move rows to batch-on-partition layout via SBUF->SBUF DMA
        PQs = work.tile([2, 2 * 512], F32, tag="PQs")
        for j in range(2):
            nc.scalar.copy(PQs[:, j * 512:(j + 1) * 512], psT[j][:, :])
        Uraw = work.tile([BPC, NT], F32, tag="Uraw")
        Sraw = work.tile([BPC, NT], F32, tag="Sraw")
        nc.sync.dma_start(Uraw[:, :], PQs[0:1, :])
        nc.sync.dma_start(Sraw[:, :], PQs[1:2, :])

        # Ulab = (Uraw + k1) * label ; Slab = (Sraw + k2) * label
        Ulab = work.tile([BPC, NT], F32, tag="Ulab")
        Slab = work.tile([BPC, NT], F32, tag="Slab")
        nc.vector.scalar_tensor_tensor(
            Ulab[:, :], Uraw[:, :], karr_sb[:, 0:1], lab_sb[:, :], AL.add, AL.mult
        )
        nc.vector.scalar_tensor_tensor(
            Slab[:, :], Sraw[:, :], karr_sb[:, 1:2], lab_sb[:, :], AL.add, AL.mult
        )

        # ---- 5-iteration scalar recurrence, batch on partitions (base 0)
        resp = work.tile([BPC, NT], F32, tag="resp")
        junk = work.tile([BPC, NT], F32, tag="junk")
        gs = [work.tile([BPC, 1], F32, tag=f"g{t}", name=f"g{t}") for t in range(NIT)]
        cs = [work.tile([BPC, 1], F32, tag=f"c{t}", name=f"c{t}") for t in range(NIT)]
        # t = 0 (ctil_0 = 0 -> resp = Ulab)
        nc.vector.scalar_tensor_tensor(
            junk[:, :], Ulab[:, :], 1.0, glm_sb[:, :], AL.is_lt, AL.mult,
            accum_out=gs[0][:, :],
        )
        nc.vector.tensor_scalar(
            out=cs[0][:, :], in0=gs[0][:, :], scalar1=float(RHO ** -1),
            scalar2=None, op0=AL.mult,
        )
        for t in range(1, NIT):
            nc.vector.scalar_tensor_tensor(
                resp[:, :], Slab[:, :], cs[t - 1][:, :], Ulab[:, :], AL.mult, AL.add
            )
            nc.vector.scalar_tensor_tensor(
                junk[:, :], resp[:, :], float(RHO ** -t), glm_sb[:, :],
                AL.is_lt, AL.mult, accum_out=gs[t][:, :],
            )
            nc.vector.scalar_tensor_tensor(
                cs[t][:, :], gs[t][:, :], float(RHO ** -(t + 1)), cs[t - 1][:, :],
                AL.mult, AL.add,
            )
        ctil5 = cs[NIT - 1]

        # ---- gather ctil5 to a row; output bias row kb = a5*k1 + (a5*k2)*ctil5
        ct5row = work.tile([1, BPC], F32, tag="ct5row")
        nc.sync.dma_start(ct5row[0:1, :], ctil5[:, :])
        kbrow = work.tile([1, BPC], F32, tag="kbrow")
        nc.vector.tensor_scalar(
            out=kbrow[0:1, :], in0=ct5row[0:1, :], scalar1=karr_sb[0:1, 3:4],
            scalar2=karr_sb[0:1, 2:3], op0=AL.mult, op1=AL.add,
        )

        # ---- broadcast ctil5 across partitions via ones-matmul; G = p + ctil5*q
        ones_t = work.tile([1, 128], F32, tag="ones_t")
        nc.vector.memset(ones_t[0:1, :], 1.0)
        c5bc = psum.tile([128, BPC], F32, tag="ps", name="c5bc")
        nc.tensor.matmul(c5bc[:, :], ones_t[0:1, :], ct5row[0:1, :],
                         start=True, stop=True)
        G = work.tile([128, KC, BPC], F32, tag="G")
        for k in range(KC):
            nc.vector.tensor_scalar(
                out=G[:, k, :], in0=c5bc[:, :], scalar1=pq_sb[:, k, 1:2],
                scalar2=pq_sb[:, k, 0:1], op0=AL.mult, op1=AL.add,
            )

        # ---- search stage: psS[b,h] (1,512) accumulates G_b^T @ xs chunks
        psS = [psum.tile([1, 512], F32, tag="ps", name=f"psS{i}")
               for i in range(2 * BPC)]
        for k in range(KC):
            for b in range(BPC):
                for h in range(2):
                    nc.tensor.matmul(
                        psS[b * 2 + h][:, :],
                        _mm(G[:, k, b:b + 1]),
                        _mm(xs_sb[k][:, b, h * 512:(h + 1) * 512]),
                        start=(k == 0),
                        stop=(k == KC - 1),
                    )

        # ---- extract: out = a5 * psum + kb[b]; stage on one partition
        out_row = work.tile([1, BPC, 2, 512], F32, tag="out_row")
        for b in range(BPC):
            for h in range(2):
                nc.scalar.activation(
                    out_row[0:1, b, h, :], psS[b * 2 + h][0:1, :],
                    AF.Identity, bias=kbrow[0:1, b:b + 1], scale=float(A5),
                )
        nc.sync.dma_start(out[:, :], out_row[0:1, :, :, :])

    nc.finalize()
    return nc


def _host_prep(inputs):
    """Host-side precomputation of p, q, k1, k2, label, glm from small weights."""
    mask = np.asarray(inputs["target_mask"], np.float32).reshape(B, NT)
    W = np.asarray(inputs["conv_w"], np.float64)
    cb = np.asarray(inputs["conv_b"], np.float64)
    gamma = np.asarray(inputs["bn_gamma"], np.float64)
    beta = np.asarray(inputs["bn_beta"], np.float64)
    mean = np.asarray(inputs["bn_mean"], np.float64)
    var = np.asarray(inputs["bn_var"], np.float64)
    f0 = np.asarray(inputs["filter_init"], np.float64).reshape(D)

    inv_std = gamma / np.sqrt(var + BN_EPS)
    cvec = (cb - mean) * inv_std + beta
    p = W.T @ (f0 * inv_std)
    q = W.T @ inv_std
    k1 = float(f0 @ cvec)
    k2 = float(cvec.sum())
    pqh = np.stack([p, q], axis=1).astype(np.float32)          # (768, 2)
    karr_row = np.array([k1, k2, A5 * k1, A5 * k2], np.float64).astype(np.float32)
    karr_h = np.broadcast_to(karr_row, (BPC, 4)).copy()

    # Gaussian label from mask centroid (float32 to mirror the fp32 reference)
    yy, xx = np.meshgrid(
        np.arange(HT, dtype=np.float32), np.arange(WT, dtype=np.float32), indexing="ij"
    )
    yf, xf = yy.reshape(-1), xx.reshape(-1)
    msum = np.maximum(mask.sum(1), np.float32(1.0))
    cy = (mask * yf).sum(1) / msum
    cx = (mask * xf).sum(1) / msum
    d2 = (xf[None, :] - cx[:, None]) ** 2 + (yf[None, :] - cy[:, None]) ** 2
    labh = np.exp(-d2 / np.float32(2.0 * SIGMA * SIGMA)).astype(np.float32)
    glmh = (np.float32(LR / NT) * labh * mask).astype(np.float32)
    return pqh, karr_h, labh, glmh


def make_in_maps(inputs):
    sf = np.ascontiguousarray(
        np.asarray(inputs["search_features"], np.float32).reshape(B, D, NS)
    )
    tf_ = np.ascontiguousarray(
        np.asarray(inputs["target_features"], np.float32).reshape(B, D, NT)
    )
    pqh, karr_h, labh, glmh = _host_prep(inputs)
    in_maps = []
    for c in range(NCORES):
        s = slice(BPC * c, BPC * (c + 1))
        in_maps.append({
            "xt": np.ascontiguousarray(tf_[s]),
            "xs": np.ascontiguousarray(sf[s]),
            "pq": pqh,
            "lab": np.ascontiguousarray(labh[s]),
            "glm": np.ascontiguousarray(glmh[s]),
            "karr": karr_h,
        })
    return in_maps


def run(inputs, trace=False, **kwargs):
    if "nc" not in _CACHE:
        _CACHE["nc"] = build()
    nc = _CACHE["nc"]
    res = run_bass_kernel_spmd(
        nc, make_in_maps(inputs), core_ids=list(range(NCORES)), trace=trace, **kwargs
    )
    outs = [res.results[c]["out"].reshape(BPC, 1, HS, WS) for c in range(NCORES)]
    return np.concatenate(outs, axis=0), res


def kernel(**inputs) -> np.ndarray:
    out, _ = run(inputs)
    return out


# revision 8
# speedup vs baseline: 1.3310x; 1.3310x over previous
"""Bass/Trainium2 kernel for nn_DiscriminativeCorrelationFilter.

Math
----
Reference computes, per batch b:
  sp = BN(W @ xs_b), tp = BN(W @ xt_b)        (1x1 conv 768->768 + eval-mode BN)
  label from mask centroid (Gaussian)
  f_0 = f_init;  5 iterations:
      r = f_t . tp  (per pixel);  cond = (r*label < 1)
      grad_b = mean(cond * (-label*mask))     (a SCALAR per batch)
      f_{t+1} = (1-LR*LAM) f_t - LR*grad_b*ones
  out_b = f_5 . sp

Because BN(W@x) = inv_std .* (W@x) + cvec (affine per channel) and f_t
stays in span{f_init, ones} (the gradient is a per-batch scalar):
  f_t = a_t * f_init + c_t * ones,  a_t = rho^t  (compile-time)
every channel contraction collapses onto two fixed vectors
    p = W^T (f_init .* inv_std),  q = W^T inv_std          (768 each)
with scalars k1 = f_init.cvec, k2 = sum(cvec):
    f_t . BN(W@x) = a_t (p^T x + k1) + c_t (q^T x + k2)
Device work per batch:
  u = p^T xt + k1, s = q^T xt + k2                      (256 each)
  recurrence on ctil_t = c_t/a_t:
    resp = u*lab + ctil_t * (s*lab); cond = resp < rho^-t
    gsum = sum(cond * glm), glm = (LR/256)*label*mask
    ctil_{t+1} = ctil_t + gsum * rho^-(t+1)
  out = a5 * (G^T xs) + (a5 k1 + a5 k2 ctil5),  G = p + ctil5 * q
p, q, k1, k2, label, glm are cheap host precomputes from the small
weights; the 126 MB of features stream through skinny matmuls once,
so the kernel is DMA-bound (~15.7 MB/core).

Sharding: data-parallel over batch, 4 batches per core on 8 cores.
Engine ops keep all SBUF operands at partition base 0 (HW requires
base in {0,32,64,96}); partition rearrangements go through small
SBUF->SBUF DMAs, which have no base restriction.
"""

import numpy as np
from contextlib import ExitStack

import concourse.bass as bass
import concourse.bacc as bacc
import concourse.mybir as mybir
import concourse.tile as tile
from concourse.bass_utils import run_bass_kernel_spmd

# ---------------- problem constants (hardcoded; kernel.py must be standalone)
B = 32            # full batch
D = 768           # feature dim
HS = WS = 32      # search spatial
HT = WT = 16      # target spatial
NS = HS * WS      # 1024
NT = HT * WT      # 256
NCORES = 8
BPC = B // NCORES  # 4 batches per core
KC = D // 128      # 6 contraction chunks

LR = 0.1
LAM = 0.01
SIGMA = 2.0
NIT = 5
BN_EPS = 1e-5
RHO = 1.0 - LR * LAM          # 0.999
A5 = RHO ** NIT

F32 = mybir.dt.float32
F32R = mybir.dt.float32r

USE_F32R = True   # stream feature matmuls as float32r (full-rate fp32 on PE)

_CACHE = {}


def _mm(ap):
    # operands already carry the f32r dtype when USE_F32R; identity otherwise
    return ap


def _f32view(ap):
    return ap.bitcast(F32) if USE_F32R else ap


FT = F32R if USE_F32R else F32   # dtype for tensors feeding the PE


def build():
    """Build the per-core Bass program (shapes only; no input values baked)."""
    nc = bacc.Bacc()
    xt = nc.dram_tensor("xt", (BPC, D, NT), FT, kind="ExternalInput")
    xs = nc.dram_tensor("xs", (BPC, D, NS), FT, kind="ExternalInput")
    pq = nc.dram_tensor("pq", (D, 2), FT, kind="ExternalInput")
    lab = nc.dram_tensor("lab", (BPC, NT), F32, kind="ExternalInput")
    glm = nc.dram_tensor("glm", (BPC, NT), F32, kind="ExternalInput")
    karr = nc.dram_tensor("karr", (BPC, 4), F32, kind="ExternalInput")
    out = nc.dram_tensor("out", (BPC, NS), F32, kind="ExternalOutput")

    AL = mybir.AluOpType
    AF = mybir.ActivationFunctionType

    with tile.TileContext(nc) as tc, ExitStack() as ctx:
        const = ctx.enter_context(tc.tile_pool(name="const", bufs=1))
        feats = ctx.enter_context(tc.tile_pool(name="feats", bufs=1))
        work = ctx.enter_context(tc.tile_pool(name="work", bufs=1))
        psum = ctx.enter_context(tc.tile_pool(name="psum", bufs=8, space="PSUM"))

        # ---- small constant loads
        pq_sb = const.tile([128, KC, 2], FT, tag="pq")
        nc.sync.dma_start(pq_sb[:, :, :], pq.rearrange("(k p) c -> p k c", p=128))
        lab_sb = const.tile([BPC, NT], F32, tag="lab")
        nc.sync.dma_start(lab_sb[:, :], lab[:, :])
        glm_sb = const.tile([BPC, NT], F32, tag="glm")
        nc.sync.dma_start(glm_sb[:, :], glm[:, :])
        karr_sb = const.tile([BPC, 4], F32, tag="karr")
        nc.sync.dma_start(karr_sb[:, :], karr[:, :])

        # ---- feature loads (target first: it gates the serial recurrence)
        xt_sb = []
        for k in range(KC):
            t = feats.tile([128, BPC, NT], FT, tag=f"xt{k}", name=f"xt{k}")
            nc.sync.dma_start(
                t[:, :, :], xt[:, k * 128:(k + 1) * 128, :].rearrange("b p n -> p b n")
            )
            xt_sb.append(t)
        xs_sb = []
        for k in range(KC):
            t = feats.tile([128, BPC, NS], FT, tag=f"xs{k}", name=f"xs{k}")
            nc.sync.dma_start(
                t[:, :, :], xs[:, k * 128:(k + 1) * 128, :].rearrange("b p n -> p b n")
            )
            xs_sb.append(t)

        # ---- target stage: psT[j] (2,512) = [p;q]^T @ xt for batches (2j, 2j+1)
        psT = [psum.tile([2, 512], F32, tag="ps", name=f"psT{j}") for j in range(2)]
        for j in range(2):
            for k in range(KC):
                nc.tensor.matmul(
                    psT[j][:, :],
                    _mm(pq_sb[:, k, :]),
                    _mm(xt_sb[k][:, 2 * j:2 * j + 2, :]),
                    start=(k == 0),
                    stop=(k == KC - 1),
                )

        # ---- move rows to batch-on-partition layout via SBUF->SBUF DMA
        PQs = work.tile([2, 2 * 512], F32, tag="PQs")
        for j in range(2):
            nc.scalar.copy(PQs[:, j * 512:(j + 1) * 512], psT[j][:, :])
        Uraw = work.tile([BPC, NT], F32, tag="Uraw")
        Sraw = work.tile([BPC, NT], F32, tag="Sraw")
        nc.sync.dma_start(Uraw[:, :], PQs[0:1, :])
        nc.sync.dma_start(Sraw[:, :], PQs[1:2, :])

        # Ulab = (Uraw + k1) * label ; Slab = (Sraw + k2) * label
        Ulab = work.tile([BPC, NT], F32, tag="Ulab")
        Slab = work.tile([BPC, NT], F32, tag="Slab")
        nc.vector.scalar_tensor_tensor(
            Ulab[:, :], Uraw[:, :], karr_sb[:, 0:1], lab_sb[:, :], AL.add, AL.mult
        )
        nc.vector.scalar_tensor_tensor(
            Slab[:, :], Sraw[:, :], karr_sb[:, 1:2], lab_sb[:, :], AL.add, AL.mult
        )

        # ---- 5-iteration scalar recurrence, batch on partitions (base 0)
        resp = work.tile([BPC, NT], F32, tag="resp")
        junk = work.tile([BPC, NT], F32, tag="junk")
        gs = [work.tile([BPC, 1], F32, tag=f"g{t}", name=f"g{t}") for t in range(NIT)]
        cs = [work.tile([BPC, 1], F32, tag=f"c{t}", name=f"c{t}") for t in range(NIT)]
        # t = 0 (ctil_0 = 0 -> resp = Ulab)
        nc.vector.scalar_tensor_tensor(
            junk[:, :], Ulab[:, :], 1.0, glm_sb[:, :], AL.is_lt, AL.mult,
            accum_out=gs[0][:, :],
        )
        nc.vector.tensor_scalar(
            out=cs[0][:, :], in0=gs[0][:, :], scalar1=float(RHO ** -1),
            scalar2=None, op0=AL.mult,
        )
        for t in range(1, NIT):
            nc.vector.scalar_tensor_tensor(
                resp[:, :], Slab[:, :], cs[t - 1][:, :], Ulab[:, :], AL.mult, AL.add
            )
            nc.vector.scalar_tensor_tensor(
                junk[:, :], resp[:, :], float(RHO ** -t), glm_sb[:, :],
                AL.is_lt, AL.mult, accum_out=gs[t][:, :],
            )
            nc.vector.scalar_tensor_tensor(
                cs[t][:, :], gs[t][:, :], float(RHO ** -(t + 1)), cs[t - 1][:, :],
                AL.mult, AL.add,
            )
        ctil5 = cs[NIT - 1]

        # ---- gather ctil5 to a row; output bias row kb = a5*k1 + (a5*k2)*ctil5
        ct5row = work.tile([1, BPC], F32, tag="ct5row")
        nc.sync.dma_start(ct5row[0:1, :], ctil5[:, :])
        kbrow = work.tile([1, BPC], F32, tag="kbrow")
        nc.vector.tensor_scalar(
            out=kbrow[0:1, :], in0=ct5row[0:1, :], scalar1=karr_sb[0:1, 3:4],
            scalar2=karr_sb[0:1, 2:3], op0=AL.mult, op1=AL.add,
        )

        # ---- broadcast ctil5 across partitions via ones-matmul; G = p + ctil5*q
        ones_t = work.tile([1, 128], F32, tag="ones_t")
        nc.vector.memset(ones_t[0:1, :], 1.0)
        c5bc = psum.tile([128, BPC], F32, tag="ps", name="c5bc")
        nc.tensor.matmul(c5bc[:, :], ones_t[0:1, :], ct5row[0:1, :],
                         start=True, stop=True)
        G = work.tile([128, KC, BPC], FT, tag="G")
        for k in range(KC):
            nc.vector.tensor_scalar(
                out=G[:, k, :], in0=c5bc[:, :], scalar1=_f32view(pq_sb[:, k, 1:2]),
                scalar2=_f32view(pq_sb[:, k, 0:1]), op0=AL.mult, op1=AL.add,
            )

        # ---- search stage: psS[b,h] (1,512) accumulates G_b^T @ xs chunks
        psS = [psum.tile([1, 512], F32, tag="ps", name=f"psS{i}")
               for i in range(2 * BPC)]
        for k in range(KC):
            for b in range(BPC):
                for h in range(2):
                    nc.tensor.matmul(
                        psS[b * 2 + h][:, :],
                        _mm(G[:, k, b:b + 1]),
                        _mm(xs_sb[k][:, b, h * 512:(h + 1) * 512]),
                        start=(k == 0),
                        stop=(k == KC - 1),
                    )

        # ---- extract: out = a5 * psum + kb[b]; stage on one partition
        out_row = work.tile([1, BPC, 2, 512], F32, tag="out_row")
        for b in range(BPC):
            for h in range(2):
                nc.scalar.activation(
                    out_row[0:1, b, h, :], psS[b * 2 + h][0:1, :],
                    AF.Identity, bias=kbrow[0:1, b:b + 1], scale=float(A5),
                )
        nc.sync.dma_start(out[:, :], out_row[0:1, :, :, :])

    nc.finalize()
    return nc


def _host_prep(inputs):
    """Host-side precomputation of p, q, k1, k2, label, glm from small weights."""
    mask = np.asarray(inputs["target_mask"], np.float32).reshape(B, NT)
    W = np.asarray(inputs["conv_w"], np.float64)
    cb = np.asarray(inputs["conv_b"], np.float64)
    gamma = np.asarray(inputs["bn_gamma"], np.float64)
    beta = np.asarray(inputs["bn_beta"], np.float64)
    mean = np.asarray(inputs["bn_mean"], np.float64)
    var = np.asarray(inputs["bn_var"], np.float64)
    f0 = np.asarray(inputs["filter_init"], np.float64).reshape(D)

    inv_std = gamma / np.sqrt(var + BN_EPS)
    cvec = (cb - mean) * inv_std + beta
    p = W.T @ (f0 * inv_std)
    q = W.T @ inv_std
    k1 = float(f0 @ cvec)
    k2 = float(cvec.sum())
    pqh = np.stack([p, q], axis=1).astype(np.float32)          # (768, 2)
    karr_row = np.array([k1, k2, A5 * k1, A5 * k2], np.float64).astype(np.float32)
    karr_h = np.broadcast_to(karr_row, (BPC, 4)).copy()

    # Gaussian label from mask centroid (float32 to mirror the fp32 reference)
    yy, xx = np.meshgrid(
        np.arange(HT, dtype=np.float32), np.arange(WT, dtype=np.float32), indexing="ij"
    )
    yf, xf = yy.reshape(-1), xx.reshape(-1)
    msum = np.maximum(mask.sum(1), np.float32(1.0))
    cy = (mask * yf).sum(1) / msum
    cx = (mask * xf).sum(1) / msum
    d2 = (xf[None, :] - cx[:, None]) ** 2 + (yf[None, :] - cy[:, None]) ** 2
    labh = np.exp(-d2 / np.float32(2.0 * SIGMA * SIGMA)).astype(np.float32)
    glmh = (np.float32(LR / NT) * labh * mask).astype(np.float32)
    return pqh, karr_h, labh, glmh


def make_in_maps(inputs):
    sf = np.ascontiguousarray(
        np.asarray(inputs["search_features"], np.float32).reshape(B, D, NS)
    )
    tf_ = np.ascontiguousarray(
        np.asarray(inputs["target_features"], np.float32).reshape(B, D, NT)
    )
    pqh, karr_h, labh, glmh = _host_prep(inputs)
    in_maps = []
    for c in range(NCORES):
        s = slice(BPC * c, BPC * (c + 1))
        in_maps.append({
            "xt": np.ascontiguousarray(tf_[s]),
            "xs": np.ascontiguousarray(sf[s]),
            "pq": pqh,
            "lab": np.ascontiguousarray(labh[s]),
            "glm": np.ascontiguousarray(glmh[s]),
            "karr": karr_h,
        })
    return in_maps


def run(inputs, trace=False, **kwargs):
    if "nc" not in _CACHE:
        _CACHE["nc"] = build()
    nc = _CACHE["nc"]
    res = run_bass_kernel_spmd(
        nc, make_in_maps(inputs), core_ids=list(range(NCORES)), trace=trace, **kwargs
    )
    outs = [res.results[c]["out"].reshape(BPC, 1, HS, WS) for c in range(NCORES)]
    return np.concatenate(outs, axis=0), res


def kernel(**inputs) -> np.ndarray:
    out, _ = run(inputs)
    return out


# revision 10
# speedup vs baseline: 1.6854x; 1.2663x over previous
"""Bass/Trainium2 kernel for nn_DiscriminativeCorrelationFilter.

Math
----
Reference computes, per batch b:
  sp = BN(W @ xs_b), tp = BN(W @ xt_b)        (1x1 conv 768->768 + eval-mode BN)
  label from mask centroid (Gaussian)
  f_0 = f_init;  5 iterations:
      r = f_t . tp  (per pixel);  cond = (r*label < 1)
      grad_b = mean(cond * (-label*mask))     (a SCALAR per batch)
      f_{t+1} = (1-LR*LAM) f_t - LR*grad_b*ones
  out_b = f_5 . sp

Because BN(W@x) = inv_std .* (W@x) + cvec (affine per channel) and f_t
stays in span{f_init, ones} (the gradient is a per-batch scalar):
  f_t = a_t * f_init + c_t * ones,  a_t = rho^t  (compile-time)
every channel contraction collapses onto two fixed vectors
    p = W^T (f_init .* inv_std),  q = W^T inv_std          (768 each)
with scalars k1 = f_init.cvec, k2 = sum(cvec):
    f_t . BN(W@x) = a_t (p^T x + k1) + c_t (q^T x + k2)
Device work per batch:
  u = p^T xt + k1, s = q^T xt + k2                      (256 each)
  recurrence on ctil_t = c_t/a_t:
    resp = u*lab + ctil_t * (s*lab); cond = resp < rho^-t
    gsum = sum(cond * glm), glm = (LR/256)*label*mask
    ctil_{t+1} = ctil_t + gsum * rho^-(t+1)
  out = a5 * (G^T xs) + (a5 k1 + a5 k2 ctil5),  G = p + ctil5 * q
p, q, k1, k2, label, glm are cheap host precomputes from the small
weights; the 126 MB of features stream through skinny matmuls once,
so the kernel is DMA-bound (~15.7 MB/core).

Sharding: data-parallel over batch, 4 batches per core on 8 cores.
Engine ops keep all SBUF operands at partition base 0 (HW requires
base in {0,32,64,96}); partition rearrangements go through small
SBUF->SBUF DMAs, which have no base restriction.
"""

import numpy as np
from contextlib import ExitStack

import concourse.bass as bass
import concourse.bacc as bacc
import concourse.mybir as mybir
import concourse.tile as tile
from concourse.bass_utils import run_bass_kernel_spmd

# ---------------- problem constants (hardcoded; kernel.py must be standalone)
B = 32            # full batch
D = 768           # feature dim
HS = WS = 32      # search spatial
HT = WT = 16      # target spatial
NS = HS * WS      # 1024
NT = HT * WT      # 256
NCORES = 8
BPC = B // NCORES  # 4 batches per core
KC = D // 128      # 6 contraction chunks

LR = 0.1
LAM = 0.01
SIGMA = 2.0
NIT = 5
BN_EPS = 1e-5
RHO = 1.0 - LR * LAM          # 0.999
A5 = RHO ** NIT

F32 = mybir.dt.float32
F32R = mybir.dt.float32r

USE_F32R = True   # stream feature matmuls as float32r (full-rate fp32 on PE)

_CACHE = {}


def _mm(ap):
    # operands already carry the f32r dtype when USE_F32R; identity otherwise
    return ap


def _f32view(ap):
    return ap.bitcast(F32) if USE_F32R else ap


FT = F32R if USE_F32R else F32   # dtype for tensors feeding the PE


def build():
    """Build the per-core Bass program (shapes only; no input values baked)."""
    nc = bacc.Bacc()
    xt = nc.dram_tensor("xt", (BPC, D, NT), FT, kind="ExternalInput")
    xs = nc.dram_tensor("xs", (BPC, D, NS), FT, kind="ExternalInput")
    pq = nc.dram_tensor("pq", (D, 2), FT, kind="ExternalInput")
    cst = nc.dram_tensor("cst", (BPC, 2 * NT + 4), F32, kind="ExternalInput")
    out = nc.dram_tensor("out", (BPC, NS), F32, kind="ExternalOutput")

    AL = mybir.AluOpType
    AF = mybir.ActivationFunctionType

    with tile.TileContext(nc) as tc, ExitStack() as ctx:
        const = ctx.enter_context(tc.tile_pool(name="const", bufs=1))
        feats = ctx.enter_context(tc.tile_pool(name="feats", bufs=1))
        work = ctx.enter_context(tc.tile_pool(name="work", bufs=1))
        psum = ctx.enter_context(tc.tile_pool(name="psum", bufs=8, space="PSUM"))

        # ---- small constant loads
        pq_sb = const.tile([128, KC, 2], FT, tag="pq")
        nc.sync.dma_start(pq_sb[:, :, :], pq.rearrange("(k p) c -> p k c", p=128))
        cst_sb = const.tile([BPC, 2 * NT + 4], F32, tag="cst")
        nc.scalar.dma_start(cst_sb[:, :], cst[:, :])
        lab_sb = cst_sb[:, 0:NT]
        glm_sb = cst_sb[:, NT:2 * NT]
        karr_sb = cst_sb[:, 2 * NT:2 * NT + 4]

        # ---- feature loads (target first: it gates the serial recurrence)
        xt_sb = []
        for k in range(KC):
            t = feats.tile([128, BPC, NT], FT, tag=f"xt{k}", name=f"xt{k}")
            nc.sync.dma_start(
                t[:, :, :], xt[:, k * 128:(k + 1) * 128, :].rearrange("b p n -> p b n")
            )
            xt_sb.append(t)
        xs_sb = []
        for k in range(KC):
            t = feats.tile([128, BPC, NS], FT, tag=f"xs{k}", name=f"xs{k}")
            nc.sync.dma_start(
                t[:, :, :], xs[:, k * 128:(k + 1) * 128, :].rearrange("b p n -> p b n")
            )
            xs_sb.append(t)

        # ---- target stage: psT[j] (2,512) = [p;q]^T @ xt for batches (2j, 2j+1)
        psT = [psum.tile([2, 512], F32, tag="ps", name=f"psT{j}") for j in range(2)]
        for j in range(2):
            for k in range(KC):
                nc.tensor.matmul(
                    psT[j][:, :],
                    _mm(pq_sb[:, k, :]),
                    _mm(xt_sb[k][:, 2 * j:2 * j + 2, :]),
                    start=(k == 0),
                    stop=(k == KC - 1),
                )

        # ---- move rows to batch-on-partition layout via SBUF->SBUF DMA
        PQs = work.tile([2, 2 * 512], F32, tag="PQs")
        for j in range(2):
            nc.scalar.copy(PQs[:, j * 512:(j + 1) * 512], psT[j][:, :])
        Uraw = work.tile([BPC, NT], F32, tag="Uraw")
        Sraw = work.tile([BPC, NT], F32, tag="Sraw")
        nc.scalar.dma_start(Uraw[:, :], PQs[0:1, :])
        nc.scalar.dma_start(Sraw[:, :], PQs[1:2, :])

        # Ulab = (Uraw + k1) * label ; Slab = (Sraw + k2) * label
        Ulab = work.tile([BPC, NT], F32, tag="Ulab")
        Slab = work.tile([BPC, NT], F32, tag="Slab")
        nc.vector.scalar_tensor_tensor(
            Ulab[:, :], Uraw[:, :], karr_sb[:, 0:1], lab_sb, AL.add, AL.mult
        )
        nc.vector.scalar_tensor_tensor(
            Slab[:, :], Sraw[:, :], karr_sb[:, 1:2], lab_sb, AL.add, AL.mult
        )

        # ---- 5-iteration scalar recurrence, batch on partitions (base 0)
        resp = work.tile([BPC, NT], F32, tag="resp")
        junk = work.tile([BPC, NT], F32, tag="junk")
        gs = [work.tile([BPC, 1], F32, tag=f"g{t}", name=f"g{t}") for t in range(NIT)]
        cs = [work.tile([BPC, 1], F32, tag=f"c{t}", name=f"c{t}") for t in range(NIT)]
        # t = 0 (ctil_0 = 0 -> resp = Ulab)
        nc.vector.scalar_tensor_tensor(
            junk[:, :], Ulab[:, :], 1.0, glm_sb, AL.is_lt, AL.mult,
            accum_out=gs[0][:, :],
        )
        nc.vector.tensor_scalar(
            out=cs[0][:, :], in0=gs[0][:, :], scalar1=float(RHO ** -1),
            scalar2=None, op0=AL.mult,
        )
        for t in range(1, NIT):
            nc.vector.scalar_tensor_tensor(
                resp[:, :], Slab[:, :], cs[t - 1][:, :], Ulab[:, :], AL.mult, AL.add
            )
            nc.vector.scalar_tensor_tensor(
                junk[:, :], resp[:, :], float(RHO ** -t), glm_sb,
                AL.is_lt, AL.mult, accum_out=gs[t][:, :],
            )
            nc.vector.scalar_tensor_tensor(
                cs[t][:, :], gs[t][:, :], float(RHO ** -(t + 1)), cs[t - 1][:, :],
                AL.mult, AL.add,
            )
        ctil5 = cs[NIT - 1]

        # ---- gather ctil5 to a row; output bias row kb = a5*k1 + (a5*k2)*ctil5
        ct5row = work.tile([1, BPC], F32, tag="ct5row")
        nc.scalar.dma_start(ct5row[0:1, :], ctil5[:, :])
        kbrow = work.tile([1, BPC], F32, tag="kbrow")
        nc.vector.tensor_scalar(
            out=kbrow[0:1, :], in0=ct5row[0:1, :], scalar1=karr_sb[0:1, 3:4],
            scalar2=karr_sb[0:1, 2:3], op0=AL.mult, op1=AL.add,
        )

        # ---- broadcast ctil5 across partitions via ones-matmul; G = p + ctil5*q
        ones_t = work.tile([1, 128], F32, tag="ones_t")
        nc.vector.memset(ones_t[0:1, :], 1.0)
        c5bc = psum.tile([128, BPC], F32, tag="ps", name="c5bc")
        nc.tensor.matmul(c5bc[:, :], ones_t[0:1, :], ct5row[0:1, :],
                         start=True, stop=True)
        G = work.tile([128, KC, BPC], FT, tag="G")
        for k in range(KC):
            nc.vector.tensor_scalar(
                out=G[:, k, :], in0=c5bc[:, :], scalar1=_f32view(pq_sb[:, k, 1:2]),
                scalar2=_f32view(pq_sb[:, k, 0:1]), op0=AL.mult, op1=AL.add,
            )

        # ---- search stage: psS[b,h] (1,512) accumulates G_b^T @ xs chunks
        psS = [psum.tile([1, 512], F32, tag="ps", name=f"psS{i}")
               for i in range(2 * BPC)]
        for k in range(KC):
            for b in range(BPC):
                for h in range(2):
                    nc.tensor.matmul(
                        psS[b * 2 + h][:, :],
                        _mm(G[:, k, b:b + 1]),
                        _mm(xs_sb[k][:, b, h * 512:(h + 1) * 512]),
                        start=(k == 0),
                        stop=(k == KC - 1),
                    )

        # ---- extract: out = a5 * psum + kb[b]; stage on one partition
        out_row = work.tile([1, BPC, 2, 512], F32, tag="out_row")
        for b in range(BPC):
            for h in range(2):
                nc.scalar.activation(
                    out_row[0:1, b, h, :], psS[b * 2 + h][0:1, :],
                    AF.Identity, bias=kbrow[0:1, b:b + 1], scale=float(A5),
                )
        nc.scalar.dma_start(out[:, :], out_row[0:1, :, :, :])

    nc.finalize()
    return nc


def _host_prep(inputs):
    """Host-side precomputation of p, q, k1, k2, label, glm from small weights."""
    mask = np.asarray(inputs["target_mask"], np.float32).reshape(B, NT)
    W = np.asarray(inputs["conv_w"], np.float64)
    cb = np.asarray(inputs["conv_b"], np.float64)
    gamma = np.asarray(inputs["bn_gamma"], np.float64)
    beta = np.asarray(inputs["bn_beta"], np.float64)
    mean = np.asarray(inputs["bn_mean"], np.float64)
    var = np.asarray(inputs["bn_var"], np.float64)
    f0 = np.asarray(inputs["filter_init"], np.float64).reshape(D)

    inv_std = gamma / np.sqrt(var + BN_EPS)
    cvec = (cb - mean) * inv_std + beta
    p = W.T @ (f0 * inv_std)
    q = W.T @ inv_std
    k1 = float(f0 @ cvec)
    k2 = float(cvec.sum())
    pqh = np.stack([p, q], axis=1).astype(np.float32)          # (768, 2)
    karr_row = np.array([k1, k2, A5 * k1, A5 * k2], np.float64).astype(np.float32)
    karr_h = np.broadcast_to(karr_row, (BPC, 4)).copy()

    # Gaussian label from mask centroid (float32 to mirror the fp32 reference)
    yy, xx = np.meshgrid(
        np.arange(HT, dtype=np.float32), np.arange(WT, dtype=np.float32), indexing="ij"
    )
    yf, xf = yy.reshape(-1), xx.reshape(-1)
    msum = np.maximum(mask.sum(1), np.float32(1.0))
    cy = (mask * yf).sum(1) / msum
    cx = (mask * xf).sum(1) / msum
    d2 = (xf[None, :] - cx[:, None]) ** 2 + (yf[None, :] - cy[:, None]) ** 2
    labh = np.exp(-d2 / np.float32(2.0 * SIGMA * SIGMA)).astype(np.float32)
    glmh = (np.float32(LR / NT) * labh * mask).astype(np.float32)
    return pqh, karr_h, labh, glmh


def make_in_maps(inputs):
    sf = np.ascontiguousarray(
        np.asarray(inputs["search_features"], np.float32).reshape(B, D, NS)
    )
    tf_ = np.ascontiguousarray(
        np.asarray(inputs["target_features"], np.float32).reshape(B, D, NT)
    )
    pqh, karr_h, labh, glmh = _host_prep(inputs)
    csth = np.concatenate(
        [labh, glmh, np.broadcast_to(karr_h[None, 0], (B, 4))], axis=1
    ).astype(np.float32)  # (B, 516)
    in_maps = []
    for c in range(NCORES):
        s = slice(BPC * c, BPC * (c + 1))
        in_maps.append({
            "xt": np.ascontiguousarray(tf_[s]),
            "xs": np.ascontiguousarray(sf[s]),
            "pq": pqh,
            "cst": np.ascontiguousarray(csth[s]),
        })
    return in_maps


def run(inputs, trace=False, **kwargs):
    if "nc" not in _CACHE:
        _CACHE["nc"] = build()
    nc = _CACHE["nc"]
    res = run_bass_kernel_spmd(
        nc, make_in_maps(inputs), core_ids=list(range(NCORES)), trace=trace, **kwargs
    )
    outs = [res.results[c]["out"].reshape(BPC, 1, HS, WS) for c in range(NCORES)]
    return np.concatenate(outs, axis=0), res


def kernel(**inputs) -> np.ndarray:
    out, _ = run(inputs)
    return out


# revision 11
# speedup vs baseline: 1.6880x; 1.0015x over previous
"""Bass/Trainium2 kernel for nn_DiscriminativeCorrelationFilter.

Math
----
Reference computes, per batch b:
  sp = BN(W @ xs_b), tp = BN(W @ xt_b)        (1x1 conv 768->768 + eval-mode BN)
  label from mask centroid (Gaussian)
  f_0 = f_init;  5 iterations:
      r = f_t . tp  (per pixel);  cond = (r*label < 1)
      grad_b = mean(cond * (-label*mask))     (a SCALAR per batch)
      f_{t+1} = (1-LR*LAM) f_t - LR*grad_b*ones
  out_b = f_5 . sp

Because BN(W@x) = inv_std .* (W@x) + cvec (affine per channel) and f_t
stays in span{f_init, ones} (the gradient is a per-batch scalar):
  f_t = a_t * f_init + c_t * ones,  a_t = rho^t  (compile-time)
every channel contraction collapses onto two fixed vectors
    p = W^T (f_init .* inv_std),  q = W^T inv_std          (768 each)
with scalars k1 = f_init.cvec, k2 = sum(cvec):
    f_t . BN(W@x) = a_t (p^T x + k1) + c_t (q^T x + k2)
Device work per batch:
  u = p^T xt + k1, s = q^T xt + k2                      (256 each)
  recurrence on ctil_t = c_t/a_t:
    resp = u*lab + ctil_t * (s*lab); cond = resp < rho^-t
    gsum = sum(cond * glm), glm = (LR/256)*label*mask
    ctil_{t+1} = ctil_t + gsum * rho^-(t+1)
  out = a5 * (G^T xs) + (a5 k1 + a5 k2 ctil5),  G = p + ctil5 * q
p, q, k1, k2, label, glm are cheap host precomputes from the small
weights; the 126 MB of features stream through skinny matmuls once,
so the kernel is DMA-bound (~15.7 MB/core).

Sharding: data-parallel over batch, 4 batches per core on 8 cores.
Engine ops keep all SBUF operands at partition base 0 (HW requires
base in {0,32,64,96}); partition rearrangements go through small
SBUF->SBUF DMAs, which have no base restriction.
"""

import numpy as np
from contextlib import ExitStack

import concourse.bass as bass
import concourse.bacc as bacc
import concourse.mybir as mybir
import concourse.tile as tile
from concourse.bass_utils import run_bass_kernel_spmd

# ---------------- problem constants (hardcoded; kernel.py must be standalone)
B = 32            # full batch
D = 768           # feature dim
HS = WS = 32      # search spatial
HT = WT = 16      # target spatial
NS = HS * WS      # 1024
NT = HT * WT      # 256
NCORES = 8
BPC = B // NCORES  # 4 batches per core
KC = D // 128      # 6 contraction chunks

LR = 0.1
LAM = 0.01
SIGMA = 2.0
NIT = 5
BN_EPS = 1e-5
RHO = 1.0 - LR * LAM          # 0.999
A5 = RHO ** NIT

F32 = mybir.dt.float32
F32R = mybir.dt.float32r

USE_F32R = True   # stream feature matmuls as float32r (full-rate fp32 on PE)

_CACHE = {}


def _mm(ap):
    # operands already carry the f32r dtype when USE_F32R; identity otherwise
    return ap


def _f32view(ap):
    return ap.bitcast(F32) if USE_F32R else ap


FT = F32R if USE_F32R else F32   # dtype for tensors feeding the PE


def build():
    """Build the per-core Bass program (shapes only; no input values baked)."""
    nc = bacc.Bacc()
    xt = nc.dram_tensor("xt", (BPC, D, NT), FT, kind="ExternalInput")
    xs = nc.dram_tensor("xs", (BPC, D, NS), FT, kind="ExternalInput")
    pq = nc.dram_tensor("pq", (D, 2), FT, kind="ExternalInput")
    cst = nc.dram_tensor("cst", (BPC, 2 * NT + 8), F32, kind="ExternalInput")
    out = nc.dram_tensor("out", (BPC, NS), F32, kind="ExternalOutput")

    AL = mybir.AluOpType
    AF = mybir.ActivationFunctionType

    with tile.TileContext(nc) as tc, ExitStack() as ctx:
        const = ctx.enter_context(tc.tile_pool(name="const", bufs=1))
        feats = ctx.enter_context(tc.tile_pool(name="feats", bufs=1))
        work = ctx.enter_context(tc.tile_pool(name="work", bufs=1))
        psum = ctx.enter_context(tc.tile_pool(name="psum", bufs=8, space="PSUM"))

        # ---- small constant loads
        pq_sb = const.tile([128, KC, 2], FT, tag="pq")
        nc.sync.dma_start(pq_sb[:, :, :], pq.rearrange("(k p) c -> p k c", p=128))
        cst_sb = const.tile([BPC, 2 * NT + 8], F32, tag="cst")
        nc.scalar.dma_start(cst_sb[:, :], cst[:, :])
        lab_sb = cst_sb[:, 0:NT]
        glm_sb = cst_sb[:, NT:2 * NT]
        karr_sb = cst_sb[:, 2 * NT:2 * NT + 4]
        i4_sb = cst_sb[:, 2 * NT + 4:2 * NT + 8]

        # ---- feature loads (target first: it gates the serial recurrence)
        xt_sb = []
        for k in range(KC):
            t = feats.tile([128, BPC, NT], FT, tag=f"xt{k}", name=f"xt{k}")
            nc.sync.dma_start(
                t[:, :, :], xt[:, k * 128:(k + 1) * 128, :].rearrange("b p n -> p b n")
            )
            xt_sb.append(t)
        xs_sb = []
        for k in range(KC):
            t = feats.tile([128, BPC, NS], FT, tag=f"xs{k}", name=f"xs{k}")
            nc.sync.dma_start(
                t[:, :, :], xs[:, k * 128:(k + 1) * 128, :].rearrange("b p n -> p b n")
            )
            xs_sb.append(t)

        # ---- target stage: psT[j] (2,512) = [p;q]^T @ xt for batches (2j, 2j+1)
        psT = [psum.tile([2, 512], F32, tag="ps", name=f"psT{j}") for j in range(2)]
        for j in range(2):
            for k in range(KC):
                nc.tensor.matmul(
                    psT[j][:, :],
                    _mm(pq_sb[:, k, :]),
                    _mm(xt_sb[k][:, 2 * j:2 * j + 2, :]),
                    start=(k == 0),
                    stop=(k == KC - 1),
                )

        # ---- move rows to batch-on-partition layout via SBUF->SBUF DMA
        PQs = work.tile([2, 2 * 512], F32, tag="PQs")
        for j in range(2):
            nc.scalar.copy(PQs[:, j * 512:(j + 1) * 512], psT[j][:, :])
        Uraw = work.tile([BPC, NT], F32, tag="Uraw")
        Sraw = work.tile([BPC, NT], F32, tag="Sraw")
        nc.scalar.dma_start(Uraw[:, :], PQs[0:1, :])
        nc.scalar.dma_start(Sraw[:, :], PQs[1:2, :])

        # Ulab = (Uraw + k1) * label ; Slab = (Sraw + k2) * label
        Ulab = work.tile([BPC, NT], F32, tag="Ulab")
        Slab = work.tile([BPC, NT], F32, tag="Slab")
        nc.vector.scalar_tensor_tensor(
            Ulab[:, :], Uraw[:, :], karr_sb[:, 0:1], lab_sb, AL.add, AL.mult
        )
        nc.vector.scalar_tensor_tensor(
            Slab[:, :], Sraw[:, :], karr_sb[:, 1:2], lab_sb, AL.add, AL.mult
        )

        # ---- 5-iteration scalar recurrence, batch on partitions (base 0)
        resp = work.tile([BPC, NT], F32, tag="resp")
        junk = work.tile([BPC, NT], F32, tag="junk")
        gs = [work.tile([BPC, 1], F32, tag=f"g{t}", name=f"g{t}") for t in range(NIT)]
        cs = [work.tile([BPC, 1], F32, tag=f"c{t}", name=f"c{t}") for t in range(NIT)]
        # t = 0 (ctil_0 = 0 -> resp = Ulab)
        nc.vector.scalar_tensor_tensor(
            junk[:, :], Ulab[:, :], 1.0, glm_sb, AL.is_lt, AL.mult,
            accum_out=gs[0][:, :],
        )
        nc.vector.tensor_scalar(
            out=cs[0][:, :], in0=gs[0][:, :], scalar1=float(RHO ** -1),
            scalar2=None, op0=AL.mult,
        )
        for t in range(1, NIT):
            nc.vector.scalar_tensor_tensor(
                resp[:, :], Slab[:, :], cs[t - 1][:, :], Ulab[:, :], AL.mult, AL.add
            )
            nc.vector.scalar_tensor_tensor(
                junk[:, :], resp[:, :], float(RHO ** -t), glm_sb,
                AL.is_lt, AL.mult, accum_out=gs[t][:, :],
            )
            nc.vector.scalar_tensor_tensor(
                cs[t][:, :], gs[t][:, :], float(RHO ** -(t + 1)), cs[t - 1][:, :],
                AL.mult, AL.add,
            )
        ctil5 = cs[NIT - 1]

        # ---- broadcast ctil5 across partitions without DMA:
        # D = diag(ctil5) via identity*ctil5, then c5bc = ones(4,128)^T @ D
        Dm = work.tile([BPC, BPC], F32, tag="Dm")
        nc.vector.tensor_scalar(
            out=Dm[:, :], in0=i4_sb, scalar1=ctil5[:, :], scalar2=None,
            op0=AL.mult,
        )
        ones4 = work.tile([BPC, 128], F32, tag="ones4")
        nc.vector.memset(ones4[:, :], 1.0)
        c5bc = psum.tile([128, BPC], F32, tag="ps", name="c5bc")
        nc.tensor.matmul(c5bc[:, :], ones4[:, :], Dm[:, :], start=True, stop=True)
        # output bias row: kb = a5*k1 + (a5*k2)*ctil5, from row 0 of c5bc
        kbrow = work.tile([1, BPC], F32, tag="kbrow")
        nc.vector.tensor_scalar(
            out=kbrow[0:1, :], in0=c5bc[0:1, :], scalar1=karr_sb[0:1, 3:4],
            scalar2=karr_sb[0:1, 2:3], op0=AL.mult, op1=AL.add,
        )
        G = work.tile([128, KC, BPC], FT, tag="G")
        for k in range(KC):
            nc.vector.tensor_scalar(
                out=G[:, k, :], in0=c5bc[:, :], scalar1=_f32view(pq_sb[:, k, 1:2]),
                scalar2=_f32view(pq_sb[:, k, 0:1]), op0=AL.mult, op1=AL.add,
            )

        # ---- search stage: psS[b,h] (1,512) accumulates G_b^T @ xs chunks
        psS = [psum.tile([1, 512], F32, tag="ps", name=f"psS{i}")
               for i in range(2 * BPC)]
        for k in range(KC):
            for b in range(BPC):
                for h in range(2):
                    nc.tensor.matmul(
                        psS[b * 2 + h][:, :],
                        _mm(G[:, k, b:b + 1]),
                        _mm(xs_sb[k][:, b, h * 512:(h + 1) * 512]),
                        start=(k == 0),
                        stop=(k == KC - 1),
                    )

        # ---- extract: out = a5 * psum + kb[b]; stage on one partition
        out_row = work.tile([1, BPC, 2, 512], F32, tag="out_row")
        for b in range(BPC):
            for h in range(2):
                if h == 0:
                    nc.scalar.activation(
                        out_row[0:1, b, h, :], psS[b * 2 + h][0:1, :],
                        AF.Identity, bias=kbrow[0:1, b:b + 1], scale=float(A5),
                    )
                else:
                    nc.vector.tensor_scalar(
                        out=out_row[0:1, b, h, :], in0=psS[b * 2 + h][0:1, :],
                        scalar1=float(A5), scalar2=kbrow[0:1, b:b + 1],
                        op0=AL.mult, op1=AL.add,
                    )
        nc.scalar.dma_start(out[:, :], out_row[0:1, :, :, :])

    nc.finalize()
    return nc


def _host_prep(inputs):
    """Host-side precomputation of p, q, k1, k2, label, glm from small weights."""
    mask = np.asarray(inputs["target_mask"], np.float32).reshape(B, NT)
    W = np.asarray(inputs["conv_w"], np.float64)
    cb = np.asarray(inputs["conv_b"], np.float64)
    gamma = np.asarray(inputs["bn_gamma"], np.float64)
    beta = np.asarray(inputs["bn_beta"], np.float64)
    mean = np.asarray(inputs["bn_mean"], np.float64)
    var = np.asarray(inputs["bn_var"], np.float64)
    f0 = np.asarray(inputs["filter_init"], np.float64).reshape(D)

    inv_std = gamma / np.sqrt(var + BN_EPS)
    cvec = (cb - mean) * inv_std + beta
    p = W.T @ (f0 * inv_std)
    q = W.T @ inv_std
    k1 = float(f0 @ cvec)
    k2 = float(cvec.sum())
    pqh = np.stack([p, q], axis=1).astype(np.float32)          # (768, 2)
    karr_row = np.array([k1, k2, A5 * k1, A5 * k2], np.float64).astype(np.float32)
    karr_h = np.broadcast_to(karr_row, (BPC, 4)).copy()

    # Gaussian label from mask centroid (float32 to mirror the fp32 reference)
    yy, xx = np.meshgrid(
        np.arange(HT, dtype=np.float32), np.arange(WT, dtype=np.float32), indexing="ij"
    )
    yf, xf = yy.reshape(-1), xx.reshape(-1)
    msum = np.maximum(mask.sum(1), np.float32(1.0))
    cy = (mask * yf).sum(1) / msum
    cx = (mask * xf).sum(1) / msum
    d2 = (xf[None, :] - cx[:, None]) ** 2 + (yf[None, :] - cy[:, None]) ** 2
    labh = np.exp(-d2 / np.float32(2.0 * SIGMA * SIGMA)).astype(np.float32)
    glmh = (np.float32(LR / NT) * labh * mask).astype(np.float32)
    return pqh, karr_h, labh, glmh


def make_in_maps(inputs):
    sf = np.ascontiguousarray(
        np.asarray(inputs["search_features"], np.float32).reshape(B, D, NS)
    )
    tf_ = np.ascontiguousarray(
        np.asarray(inputs["target_features"], np.float32).reshape(B, D, NT)
    )
    pqh, karr_h, labh, glmh = _host_prep(inputs)
    i4h = np.broadcast_to(np.eye(BPC, dtype=np.float32)[None], (NCORES, BPC, BPC))
    csth = np.concatenate(
        [labh, glmh, np.broadcast_to(karr_h[None, 0], (B, 4)),
         i4h.reshape(B, BPC)], axis=1
    ).astype(np.float32)  # (B, 520)
    in_maps = []
    for c in range(NCORES):
        s = slice(BPC * c, BPC * (c + 1))
        in_maps.append({
            "xt": np.ascontiguousarray(tf_[s]),
            "xs": np.ascontiguousarray(sf[s]),
            "pq": pqh,
            "cst": np.ascontiguousarray(csth[s]),
        })
    return in_maps


def run(inputs, trace=False, **kwargs):
    if "nc" not in _CACHE:
        _CACHE["nc"] = build()
    nc = _CACHE["nc"]
    res = run_bass_kernel_spmd(
        nc, make_in_maps(inputs), core_ids=list(range(NCORES)), trace=trace, **kwargs
    )
    outs = [res.results[c]["out"].reshape(BPC, 1, HS, WS) for c in range(NCORES)]
    return np.concatenate(outs, axis=0), res


def kernel(**inputs) -> np.ndarray:
    out, _ = run(inputs)
    return out


# revision 12
# speedup vs baseline: 1.9903x; 1.1791x over previous
"""Bass/Trainium2 kernel for nn_DiscriminativeCorrelationFilter.

Math
----
Reference computes, per batch b:
  sp = BN(W @ xs_b), tp = BN(W @ xt_b)        (1x1 conv 768->768 + eval-mode BN)
  label from mask centroid (Gaussian)
  f_0 = f_init;  5 iterations:
      r = f_t . tp  (per pixel);  cond = (r*label < 1)
      grad_b = mean(cond * (-label*mask))     (a SCALAR per batch)
      f_{t+1} = (1-LR*LAM) f_t - LR*grad_b*ones
  out_b = f_5 . sp

Because BN(W@x) = inv_std .* (W@x) + cvec (affine per channel) and f_t
stays in span{f_init, ones} (the gradient is a per-batch scalar):
  f_t = a_t * f_init + c_t * ones,  a_t = rho^t  (compile-time)
every channel contraction collapses onto two fixed vectors
    p = W^T (f_init .* inv_std),  q = W^T inv_std          (768 each)
with scalars k1 = f_init.cvec, k2 = sum(cvec):
    f_t . BN(W@x) = a_t (p^T x + k1) + c_t (q^T x + k2)
Device work per batch:
  u = p^T xt + k1, s = q^T xt + k2                      (256 each)
  recurrence on ctil_t = c_t/a_t:
    resp = u*lab + ctil_t * (s*lab); cond = resp < rho^-t
    gsum = sum(cond * glm), glm = (LR/256)*label*mask
    ctil_{t+1} = ctil_t + gsum * rho^-(t+1)
  out = a5 * (G^T xs) + (a5 k1 + a5 k2 ctil5),  G = p + ctil5 * q
p, q, k1, k2, label, glm are cheap host precomputes from the small
weights; the 126 MB of features stream through skinny matmuls once,
so the kernel is DMA-bound (~15.7 MB/core).

Sharding: data-parallel over batch, 4 batches per core on 8 cores.
Engine ops keep all SBUF operands at partition base 0 (HW requires
base in {0,32,64,96}); partition rearrangements go through small
SBUF->SBUF DMAs, which have no base restriction.
"""

import numpy as np
import ml_dtypes
from contextlib import ExitStack

import concourse.bass as bass
import concourse.bacc as bacc
import concourse.mybir as mybir
import concourse.tile as tile
from concourse.bass_utils import run_bass_kernel_spmd

# ---------------- problem constants (hardcoded; kernel.py must be standalone)
B = 32            # full batch
D = 768           # feature dim
HS = WS = 32      # search spatial
HT = WT = 16      # target spatial
NS = HS * WS      # 1024
NT = HT * WT      # 256
NCORES = 8
BPC = B // NCORES  # 4 batches per core
KC = D // 128      # 6 contraction chunks

LR = 0.1
LAM = 0.01
SIGMA = 2.0
NIT = 5
BN_EPS = 1e-5
RHO = 1.0 - LR * LAM          # 0.999
A5 = RHO ** NIT

F32 = mybir.dt.float32
F32R = mybir.dt.float32r
BF16 = mybir.dt.bfloat16

USE_F32R = True   # stream target-feature matmuls as float32r (full-rate fp32)
USE_BF16_XS = True  # stream search features as bf16 (halves the dominant DMA)

_CACHE = {}


def _mm(ap):
    # operands already carry the f32r dtype when USE_F32R; identity otherwise
    return ap


def _f32view(ap):
    return ap.bitcast(F32) if USE_F32R else ap


FT = F32R if USE_F32R else F32   # dtype for target-path PE tensors
XT_ = FT
XS_DT = BF16 if USE_BF16_XS else FT


def build():
    """Build the per-core Bass program (shapes only; no input values baked)."""
    nc = bacc.Bacc()
    xt = nc.dram_tensor("xt", (BPC, D, NT), FT, kind="ExternalInput")
    xs = nc.dram_tensor("xs", (BPC, D, NS), XS_DT, kind="ExternalInput")
    pq = nc.dram_tensor("pq", (D, 2), FT, kind="ExternalInput")
    cst = nc.dram_tensor("cst", (BPC, 2 * NT + 8), F32, kind="ExternalInput")
    out = nc.dram_tensor("out", (BPC, NS), F32, kind="ExternalOutput")

    AL = mybir.AluOpType
    AF = mybir.ActivationFunctionType

    with tile.TileContext(nc) as tc, ExitStack() as ctx:
        const = ctx.enter_context(tc.tile_pool(name="const", bufs=1))
        feats = ctx.enter_context(tc.tile_pool(name="feats", bufs=1))
        work = ctx.enter_context(tc.tile_pool(name="work", bufs=1))
        psum = ctx.enter_context(tc.tile_pool(name="psum", bufs=8, space="PSUM"))

        # ---- small constant loads
        pq_sb = const.tile([128, KC, 2], FT, tag="pq")
        nc.sync.dma_start(pq_sb[:, :, :], pq.rearrange("(k p) c -> p k c", p=128))
        cst_sb = const.tile([BPC, 2 * NT + 8], F32, tag="cst")
        nc.scalar.dma_start(cst_sb[:, :], cst[:, :])
        lab_sb = cst_sb[:, 0:NT]
        glm_sb = cst_sb[:, NT:2 * NT]
        karr_sb = cst_sb[:, 2 * NT:2 * NT + 4]
        i4_sb = cst_sb[:, 2 * NT + 4:2 * NT + 8]

        # ---- feature loads (target first: it gates the serial recurrence)
        xt_sb = []
        for k in range(KC):
            t = feats.tile([128, BPC, NT], FT, tag=f"xt{k}", name=f"xt{k}")
            nc.sync.dma_start(
                t[:, :, :], xt[:, k * 128:(k + 1) * 128, :].rearrange("b p n -> p b n")
            )
            xt_sb.append(t)
        xs_sb = []
        for k in range(KC):
            t = feats.tile([128, BPC, NS], XS_DT, tag=f"xs{k}", name=f"xs{k}")
            nc.sync.dma_start(
                t[:, :, :], xs[:, k * 128:(k + 1) * 128, :].rearrange("b p n -> p b n")
            )
            xs_sb.append(t)

        # ---- target stage: psT[j] (2,512) = [p;q]^T @ xt for batches (2j, 2j+1)
        psT = [psum.tile([2, 512], F32, tag="ps", name=f"psT{j}") for j in range(2)]
        for j in range(2):
            for k in range(KC):
                nc.tensor.matmul(
                    psT[j][:, :],
                    _mm(pq_sb[:, k, :]),
                    _mm(xt_sb[k][:, 2 * j:2 * j + 2, :]),
                    start=(k == 0),
                    stop=(k == KC - 1),
                )

        # ---- move rows to batch-on-partition layout via SBUF->SBUF DMA
        PQs = work.tile([2, 2 * 512], F32, tag="PQs")
        for j in range(2):
            nc.scalar.copy(PQs[:, j * 512:(j + 1) * 512], psT[j][:, :])
        Uraw = work.tile([BPC, NT], F32, tag="Uraw")
        Sraw = work.tile([BPC, NT], F32, tag="Sraw")
        nc.scalar.dma_start(Uraw[:, :], PQs[0:1, :])
        nc.scalar.dma_start(Sraw[:, :], PQs[1:2, :])

        # Ulab = (Uraw + k1) * label ; Slab = (Sraw + k2) * label
        Ulab = work.tile([BPC, NT], F32, tag="Ulab")
        Slab = work.tile([BPC, NT], F32, tag="Slab")
        nc.vector.scalar_tensor_tensor(
            Ulab[:, :], Uraw[:, :], karr_sb[:, 0:1], lab_sb, AL.add, AL.mult
        )
        nc.vector.scalar_tensor_tensor(
            Slab[:, :], Sraw[:, :], karr_sb[:, 1:2], lab_sb, AL.add, AL.mult
        )

        # ---- 5-iteration scalar recurrence, batch on partitions (base 0)
        resp = work.tile([BPC, NT], F32, tag="resp")
        junk = work.tile([BPC, NT], F32, tag="junk")
        gs = [work.tile([BPC, 1], F32, tag=f"g{t}", name=f"g{t}") for t in range(NIT)]
        cs = [work.tile([BPC, 1], F32, tag=f"c{t}", name=f"c{t}") for t in range(NIT)]
        # t = 0 (ctil_0 = 0 -> resp = Ulab)
        nc.vector.scalar_tensor_tensor(
            junk[:, :], Ulab[:, :], 1.0, glm_sb, AL.is_lt, AL.mult,
            accum_out=gs[0][:, :],
        )
        nc.vector.tensor_scalar(
            out=cs[0][:, :], in0=gs[0][:, :], scalar1=float(RHO ** -1),
            scalar2=None, op0=AL.mult,
        )
        for t in range(1, NIT):
            nc.vector.scalar_tensor_tensor(
                resp[:, :], Slab[:, :], cs[t - 1][:, :], Ulab[:, :], AL.mult, AL.add
            )
            nc.vector.scalar_tensor_tensor(
                junk[:, :], resp[:, :], float(RHO ** -t), glm_sb,
                AL.is_lt, AL.mult, accum_out=gs[t][:, :],
            )
            nc.vector.scalar_tensor_tensor(
                cs[t][:, :], gs[t][:, :], float(RHO ** -(t + 1)), cs[t - 1][:, :],
                AL.mult, AL.add,
            )
        ctil5 = cs[NIT - 1]

        # ---- broadcast ctil5 across partitions without DMA:
        # D = diag(ctil5) via identity*ctil5, then c5bc = ones(4,128)^T @ D
        Dm = work.tile([BPC, BPC], F32, tag="Dm")
        nc.vector.tensor_scalar(
            out=Dm[:, :], in0=i4_sb, scalar1=ctil5[:, :], scalar2=None,
            op0=AL.mult,
        )
        ones4 = work.tile([BPC, 128], F32, tag="ones4")
        nc.vector.memset(ones4[:, :], 1.0)
        c5bc = psum.tile([128, BPC], F32, tag="ps", name="c5bc")
        nc.tensor.matmul(c5bc[:, :], ones4[:, :], Dm[:, :], start=True, stop=True)
        # output bias row: kb = a5*k1 + (a5*k2)*ctil5, from row 0 of c5bc
        kbrow = work.tile([1, BPC], F32, tag="kbrow")
        nc.vector.tensor_scalar(
            out=kbrow[0:1, :], in0=c5bc[0:1, :], scalar1=karr_sb[0:1, 3:4],
            scalar2=karr_sb[0:1, 2:3], op0=AL.mult, op1=AL.add,
        )
        G = work.tile([128, KC, BPC], XS_DT, tag="G")
        for k in range(KC):
            nc.vector.tensor_scalar(
                out=G[:, k, :], in0=c5bc[:, :], scalar1=_f32view(pq_sb[:, k, 1:2]),
                scalar2=_f32view(pq_sb[:, k, 0:1]), op0=AL.mult, op1=AL.add,
            )

        # ---- search stage: psS[b,h] (1,512) accumulates G_b^T @ xs chunks
        psS = [psum.tile([1, 512], F32, tag="ps", name=f"psS{i}")
               for i in range(2 * BPC)]
        for k in range(KC):
            for b in range(BPC):
                for h in range(2):
                    nc.tensor.matmul(
                        psS[b * 2 + h][:, :],
                        _mm(G[:, k, b:b + 1]),
                        _mm(xs_sb[k][:, b, h * 512:(h + 1) * 512]),
                        start=(k == 0),
                        stop=(k == KC - 1),
                    )

        # ---- extract: out = a5 * psum + kb[b]; stage on one partition
        out_row = work.tile([1, BPC, 2, 512], F32, tag="out_row")
        for b in range(BPC):
            for h in range(2):
                if h == 0:
                    nc.scalar.activation(
                        out_row[0:1, b, h, :], psS[b * 2 + h][0:1, :],
                        AF.Identity, bias=kbrow[0:1, b:b + 1], scale=float(A5),
                    )
                else:
                    nc.vector.tensor_scalar(
                        out=out_row[0:1, b, h, :], in0=psS[b * 2 + h][0:1, :],
                        scalar1=float(A5), scalar2=kbrow[0:1, b:b + 1],
                        op0=AL.mult, op1=AL.add,
                    )
        nc.scalar.dma_start(out[:, :], out_row[0:1, :, :, :])

    nc.finalize()
    return nc


def _host_prep(inputs):
    """Host-side precomputation of p, q, k1, k2, label, glm from small weights."""
    mask = np.asarray(inputs["target_mask"], np.float32).reshape(B, NT)
    W = np.asarray(inputs["conv_w"], np.float64)
    cb = np.asarray(inputs["conv_b"], np.float64)
    gamma = np.asarray(inputs["bn_gamma"], np.float64)
    beta = np.asarray(inputs["bn_beta"], np.float64)
    mean = np.asarray(inputs["bn_mean"], np.float64)
    var = np.asarray(inputs["bn_var"], np.float64)
    f0 = np.asarray(inputs["filter_init"], np.float64).reshape(D)

    inv_std = gamma / np.sqrt(var + BN_EPS)
    cvec = (cb - mean) * inv_std + beta
    p = W.T @ (f0 * inv_std)
    q = W.T @ inv_std
    k1 = float(f0 @ cvec)
    k2 = float(cvec.sum())
    pqh = np.stack([p, q], axis=1).astype(np.float32)          # (768, 2)
    karr_row = np.array([k1, k2, A5 * k1, A5 * k2], np.float64).astype(np.float32)
    karr_h = np.broadcast_to(karr_row, (BPC, 4)).copy()

    # Gaussian label from mask centroid (float32 to mirror the fp32 reference)
    yy, xx = np.meshgrid(
        np.arange(HT, dtype=np.float32), np.arange(WT, dtype=np.float32), indexing="ij"
    )
    yf, xf = yy.reshape(-1), xx.reshape(-1)
    msum = np.maximum(mask.sum(1), np.float32(1.0))
    cy = (mask * yf).sum(1) / msum
    cx = (mask * xf).sum(1) / msum
    d2 = (xf[None, :] - cx[:, None]) ** 2 + (yf[None, :] - cy[:, None]) ** 2
    labh = np.exp(-d2 / np.float32(2.0 * SIGMA * SIGMA)).astype(np.float32)
    glmh = (np.float32(LR / NT) * labh * mask).astype(np.float32)
    return pqh, karr_h, labh, glmh


def make_in_maps(inputs):
    sf = np.asarray(inputs["search_features"], np.float32).reshape(B, D, NS)
    if USE_BF16_XS:
        sf = sf.astype(ml_dtypes.bfloat16)
    sf = np.ascontiguousarray(sf)
    tf_ = np.ascontiguousarray(
        np.asarray(inputs["target_features"], np.float32).reshape(B, D, NT)
    )
    pqh, karr_h, labh, glmh = _host_prep(inputs)
    i4h = np.broadcast_to(np.eye(BPC, dtype=np.float32)[None], (NCORES, BPC, BPC))
    csth = np.concatenate(
        [labh, glmh, np.broadcast_to(karr_h[None, 0], (B, 4)),
         i4h.reshape(B, BPC)], axis=1
    ).astype(np.float32)  # (B, 520)
    in_maps = []
    for c in range(NCORES):
        s = slice(BPC * c, BPC * (c + 1))
        in_maps.append({
            "xt": np.ascontiguousarray(tf_[s]),
            "xs": np.ascontiguousarray(sf[s]),
            "pq": pqh,
            "cst": np.ascontiguousarray(csth[s]),
        })
    return in_maps


def run(inputs, trace=False, **kwargs):
    if "nc" not in _CACHE:
        _CACHE["nc"] = build()
    nc = _CACHE["nc"]
    res = run_bass_kernel_spmd(
        nc, make_in_maps(inputs), core_ids=list(range(NCORES)), trace=trace, **kwargs
    )
    outs = [res.results[c]["out"].reshape(BPC, 1, HS, WS) for c in range(NCORES)]
    return np.concatenate(outs, axis=0), res


def kernel(**inputs) -> np.ndarray:
    out, _ = run(inputs)
    return out


# revision 13
# speedup vs baseline: 2.0571x; 1.0336x over previous
"""Bass/Trainium2 kernel for nn_DiscriminativeCorrelationFilter.

Math
----
Reference computes, per batch b:
  sp = BN(W @ xs_b), tp = BN(W @ xt_b)        (1x1 conv 768->768 + eval-mode BN)
  label from mask centroid (Gaussian)
  f_0 = f_init;  5 iterations:
      r = f_t . tp  (per pixel);  cond = (r*label < 1)
      grad_b = mean(cond * (-label*mask))     (a SCALAR per batch)
      f_{t+1} = (1-LR*LAM) f_t - LR*grad_b*ones
  out_b = f_5 . sp

Because BN(W@x) = inv_std .* (W@x) + cvec (affine per channel) and f_t
stays in span{f_init, ones} (the gradient is a per-batch scalar):
  f_t = a_t * f_init + c_t * ones,  a_t = rho^t  (compile-time)
every channel contraction collapses onto two fixed vectors
    p = W^T (f_init .* inv_std),  q = W^T inv_std          (768 each)
with scalars k1 = f_init.cvec, k2 = sum(cvec):
    f_t . BN(W@x) = a_t (p^T x + k1) + c_t (q^T x + k2)
Device work per batch:
  u = p^T xt + k1, s = q^T xt + k2                      (256 each)
  recurrence on ctil_t = c_t/a_t:
    resp = u*lab + ctil_t * (s*lab); cond = resp < rho^-t
    gsum = sum(cond * glm), glm = (LR/256)*label*mask
    ctil_{t+1} = ctil_t + gsum * rho^-(t+1)
  out = a5 * (G^T xs) + (a5 k1 + a5 k2 ctil5),  G = p + ctil5 * q
p, q, k1, k2, label, glm are cheap host precomputes from the small
weights; the 126 MB of features stream through skinny matmuls once,
so the kernel is DMA-bound (~15.7 MB/core).

Sharding: data-parallel over batch, 4 batches per core on 8 cores.
Engine ops keep all SBUF operands at partition base 0 (HW requires
base in {0,32,64,96}); partition rearrangements go through small
SBUF->SBUF DMAs, which have no base restriction.
"""

import numpy as np
import ml_dtypes
from contextlib import ExitStack

import concourse.bass as bass
import concourse.bacc as bacc
import concourse.mybir as mybir
import concourse.tile as tile
from concourse.bass_utils import run_bass_kernel_spmd

# ---------------- problem constants (hardcoded; kernel.py must be standalone)
B = 32            # full batch
D = 768           # feature dim
HS = WS = 32      # search spatial
HT = WT = 16      # target spatial
NS = HS * WS      # 1024
NT = HT * WT      # 256
NCORES = 8
BPC = B // NCORES  # 4 batches per core
KC = D // 128      # 6 contraction chunks

LR = 0.1
LAM = 0.01
SIGMA = 2.0
NIT = 5
BN_EPS = 1e-5
RHO = 1.0 - LR * LAM          # 0.999
A5 = RHO ** NIT

F32 = mybir.dt.float32
F32R = mybir.dt.float32r
BF16 = mybir.dt.bfloat16

USE_F32R = True   # stream target-feature matmuls as float32r (full-rate fp32)
USE_BF16_XS = True  # stream search features as bf16 (halves the dominant DMA)

_CACHE = {}


def _mm(ap):
    # operands already carry the f32r dtype when USE_F32R; identity otherwise
    return ap


def _f32view(ap):
    return ap.bitcast(F32) if USE_F32R else ap


FT = F32R if USE_F32R else F32   # dtype for target-path PE tensors
XT_ = FT
XS_DT = BF16 if USE_BF16_XS else FT


def build():
    """Build the per-core Bass program (shapes only; no input values baked)."""
    nc = bacc.Bacc()
    xt = nc.dram_tensor("xt", (BPC, D, NT), FT, kind="ExternalInput")
    xs = nc.dram_tensor("xs", (BPC, D, NS), XS_DT, kind="ExternalInput")
    pq = nc.dram_tensor("pq", (D, 2), FT, kind="ExternalInput")
    cst = nc.dram_tensor("cst", (BPC, 2 * NT + 40), F32, kind="ExternalInput")
    out = nc.dram_tensor("out", (BPC, NS), F32, kind="ExternalOutput")

    AL = mybir.AluOpType
    AF = mybir.ActivationFunctionType

    with tile.TileContext(nc) as tc, ExitStack() as ctx:
        const = ctx.enter_context(tc.tile_pool(name="const", bufs=1))
        feats = ctx.enter_context(tc.tile_pool(name="feats", bufs=1))
        work = ctx.enter_context(tc.tile_pool(name="work", bufs=1))
        psum = ctx.enter_context(tc.tile_pool(name="psum", bufs=8, space="PSUM"))

        # ---- small constant loads
        pq_sb = const.tile([128, KC, 2], FT, tag="pq")
        nc.sync.dma_start(pq_sb[:, :, :], pq.rearrange("(k p) c -> p k c", p=128))
        cst_sb = const.tile([BPC, 2 * NT + 40], F32, tag="cst")
        nc.scalar.dma_start(cst_sb[:, :], cst[:, :])
        lab_sb = cst_sb[:, 0:NT]
        glm_sb = cst_sb[:, NT:2 * NT]
        karr_sb = cst_sb[:, 2 * NT:2 * NT + 4]
        i4_sb = cst_sb[:, 2 * NT + 4:2 * NT + 8]
        selu_sb = cst_sb[:, 2 * NT + 8:2 * NT + 24]
        sels_sb = cst_sb[:, 2 * NT + 24:2 * NT + 40]

        # ---- feature loads (target first: it gates the serial recurrence)
        xt_sb = []
        for k in range(KC):
            t = feats.tile([128, BPC, NT], FT, tag=f"xt{k}", name=f"xt{k}")
            nc.sync.dma_start(
                t[:, :, :], xt[:, k * 128:(k + 1) * 128, :].rearrange("b p n -> p b n")
            )
            xt_sb.append(t)
        xs_sb = []
        for k in range(KC):
            t = feats.tile([128, BPC, NS], XS_DT, tag=f"xs{k}", name=f"xs{k}")
            nc.sync.dma_start(
                t[:, :, :], xs[:, k * 128:(k + 1) * 128, :].rearrange("b p n -> p b n")
            )
            xs_sb.append(t)

        # ---- target stage: psT[j] (2,512) = [p;q]^T @ xt for batches (2j, 2j+1)
        psT = [psum.tile([2, 512], F32, tag="ps", name=f"psT{j}") for j in range(2)]
        for j in range(2):
            for k in range(KC):
                nc.tensor.matmul(
                    psT[j][:, :],
                    _mm(pq_sb[:, k, :]),
                    _mm(xt_sb[k][:, 2 * j:2 * j + 2, :]),
                    start=(k == 0),
                    stop=(k == KC - 1),
                )

        # ---- move rows to batch-on-partition layout via SBUF->SBUF DMA
        PQs = work.tile([2, 2 * 512], F32, tag="PQs")
        for j in range(2):
            nc.scalar.copy(PQs[:, j * 512:(j + 1) * 512], psT[j][:, :])
        # selection matmuls transpose the psT rows into batch-on-partition
        # PSUM tiles (no SBUF->SBUF DMA latency): psU[b, :] = PQs[0, b-block]
        psU = psum.tile([BPC, NT], F32, tag="ps", name="psU")
        psSv = psum.tile([BPC, NT], F32, tag="ps", name="psSv")
        for b in range(BPC):
            nc.tensor.matmul(
                psU[:, :], selu_sb[0:2, 4 * b:4 * b + 4],
                PQs[0:2, b * NT:(b + 1) * NT],
                start=(b == 0), stop=(b == BPC - 1),
            )
        for b in range(BPC):
            nc.tensor.matmul(
                psSv[:, :], sels_sb[0:2, 4 * b:4 * b + 4],
                PQs[0:2, b * NT:(b + 1) * NT],
                start=(b == 0), stop=(b == BPC - 1),
            )

        # Ulab = (psU + k1) * label ; Slab = (psSv + k2) * label
        Ulab = work.tile([BPC, NT], F32, tag="Ulab")
        Slab = work.tile([BPC, NT], F32, tag="Slab")
        nc.vector.scalar_tensor_tensor(
            Ulab[:, :], psU[:, :], karr_sb[:, 0:1], lab_sb, AL.add, AL.mult
        )
        nc.vector.scalar_tensor_tensor(
            Slab[:, :], psSv[:, :], karr_sb[:, 1:2], lab_sb, AL.add, AL.mult
        )

        # ---- 5-iteration scalar recurrence, batch on partitions (base 0)
        resp = work.tile([BPC, NT], F32, tag="resp")
        junk = work.tile([BPC, NT], F32, tag="junk")
        gs = [work.tile([BPC, 1], F32, tag=f"g{t}", name=f"g{t}") for t in range(NIT)]
        cs = [work.tile([BPC, 1], F32, tag=f"c{t}", name=f"c{t}") for t in range(NIT)]
        # t = 0 (ctil_0 = 0 -> resp = Ulab)
        nc.vector.scalar_tensor_tensor(
            junk[:, :], Ulab[:, :], 1.0, glm_sb, AL.is_lt, AL.mult,
            accum_out=gs[0][:, :],
        )
        nc.vector.tensor_scalar(
            out=cs[0][:, :], in0=gs[0][:, :], scalar1=float(RHO ** -1),
            scalar2=None, op0=AL.mult,
        )
        for t in range(1, NIT):
            nc.vector.scalar_tensor_tensor(
                resp[:, :], Slab[:, :], cs[t - 1][:, :], Ulab[:, :], AL.mult, AL.add
            )
            nc.vector.scalar_tensor_tensor(
                junk[:, :], resp[:, :], float(RHO ** -t), glm_sb,
                AL.is_lt, AL.mult, accum_out=gs[t][:, :],
            )
            nc.vector.scalar_tensor_tensor(
                cs[t][:, :], gs[t][:, :], float(RHO ** -(t + 1)), cs[t - 1][:, :],
                AL.mult, AL.add,
            )
        ctil5 = cs[NIT - 1]

        # ---- broadcast ctil5 across partitions without DMA:
        # D = diag(ctil5) via identity*ctil5, then c5bc = ones(4,128)^T @ D
        Dm = work.tile([BPC, BPC], F32, tag="Dm")
        nc.vector.tensor_scalar(
            out=Dm[:, :], in0=i4_sb, scalar1=ctil5[:, :], scalar2=None,
            op0=AL.mult,
        )
        ones4 = work.tile([BPC, 128], F32, tag="ones4")
        nc.vector.memset(ones4[:, :], 1.0)
        c5bc = psum.tile([128, BPC], F32, tag="ps", name="c5bc")
        nc.tensor.matmul(c5bc[:, :], ones4[:, :], Dm[:, :], start=True, stop=True)
        # output bias row: kb = a5*k1 + (a5*k2)*ctil5, from row 0 of c5bc
        kbrow = work.tile([1, BPC], F32, tag="kbrow")
        nc.vector.tensor_scalar(
            out=kbrow[0:1, :], in0=c5bc[0:1, :], scalar1=karr_sb[0:1, 3:4],
            scalar2=karr_sb[0:1, 2:3], op0=AL.mult, op1=AL.add,
        )
        G = work.tile([128, KC, BPC], XS_DT, tag="G")
        for k in range(KC):
            nc.vector.tensor_scalar(
                out=G[:, k, :], in0=c5bc[:, :], scalar1=_f32view(pq_sb[:, k, 1:2]),
                scalar2=_f32view(pq_sb[:, k, 0:1]), op0=AL.mult, op1=AL.add,
            )

        # ---- search stage: psS[b,h] (1,512) accumulates G_b^T @ xs chunks
        psS = [psum.tile([1, 512], F32, tag="ps", name=f"psS{i}")
               for i in range(2 * BPC)]
        for k in range(KC):
            for b in range(BPC):
                for h in range(2):
                    nc.tensor.matmul(
                        psS[b * 2 + h][:, :],
                        _mm(G[:, k, b:b + 1]),
                        _mm(xs_sb[k][:, b, h * 512:(h + 1) * 512]),
                        start=(k == 0),
                        stop=(k == KC - 1),
                    )

        # ---- extract: out = a5 * psum + kb[b]; stage on one partition
        out_row = work.tile([1, BPC, 2, 512], F32, tag="out_row")
        for b in range(BPC):
            for h in range(2):
                if h == 0:
                    nc.scalar.activation(
                        out_row[0:1, b, h, :], psS[b * 2 + h][0:1, :],
                        AF.Identity, bias=kbrow[0:1, b:b + 1], scale=float(A5),
                    )
                else:
                    nc.vector.tensor_scalar(
                        out=out_row[0:1, b, h, :], in0=psS[b * 2 + h][0:1, :],
                        scalar1=float(A5), scalar2=kbrow[0:1, b:b + 1],
                        op0=AL.mult, op1=AL.add,
                    )
        nc.scalar.dma_start(out[:, :], out_row[0:1, :, :, :])

    nc.finalize()
    return nc


def _host_prep(inputs):
    """Host-side precomputation of p, q, k1, k2, label, glm from small weights."""
    mask = np.asarray(inputs["target_mask"], np.float32).reshape(B, NT)
    W = np.asarray(inputs["conv_w"], np.float64)
    cb = np.asarray(inputs["conv_b"], np.float64)
    gamma = np.asarray(inputs["bn_gamma"], np.float64)
    beta = np.asarray(inputs["bn_beta"], np.float64)
    mean = np.asarray(inputs["bn_mean"], np.float64)
    var = np.asarray(inputs["bn_var"], np.float64)
    f0 = np.asarray(inputs["filter_init"], np.float64).reshape(D)

    inv_std = gamma / np.sqrt(var + BN_EPS)
    cvec = (cb - mean) * inv_std + beta
    p = W.T @ (f0 * inv_std)
    q = W.T @ inv_std
    k1 = float(f0 @ cvec)
    k2 = float(cvec.sum())
    pqh = np.stack([p, q], axis=1).astype(np.float32)          # (768, 2)
    karr_row = np.array([k1, k2, A5 * k1, A5 * k2], np.float64).astype(np.float32)
    karr_h = np.broadcast_to(karr_row, (BPC, 4)).copy()

    # Gaussian label from mask centroid (float32 to mirror the fp32 reference)
    yy, xx = np.meshgrid(
        np.arange(HT, dtype=np.float32), np.arange(WT, dtype=np.float32), indexing="ij"
    )
    yf, xf = yy.reshape(-1), xx.reshape(-1)
    msum = np.maximum(mask.sum(1), np.float32(1.0))
    cy = (mask * yf).sum(1) / msum
    cx = (mask * xf).sum(1) / msum
    d2 = (xf[None, :] - cx[:, None]) ** 2 + (yf[None, :] - cy[:, None]) ** 2
    labh = np.exp(-d2 / np.float32(2.0 * SIGMA * SIGMA)).astype(np.float32)
    glmh = (np.float32(LR / NT) * labh * mask).astype(np.float32)
    return pqh, karr_h, labh, glmh


def make_in_maps(inputs):
    sf = np.asarray(inputs["search_features"], np.float32).reshape(B, D, NS)
    if USE_BF16_XS:
        sf = sf.astype(ml_dtypes.bfloat16)
    sf = np.ascontiguousarray(sf)
    tf_ = np.ascontiguousarray(
        np.asarray(inputs["target_features"], np.float32).reshape(B, D, NT)
    )
    pqh, karr_h, labh, glmh = _host_prep(inputs)
    i4h = np.broadcast_to(np.eye(BPC, dtype=np.float32)[None], (NCORES, BPC, BPC))
    # selection matrices, rows 0-1 meaningful: selU[0, 4b+m] = (m == b)
    selu = np.zeros((BPC, 4 * BPC), np.float32)
    sels = np.zeros((BPC, 4 * BPC), np.float32)
    for b in range(BPC):
        selu[0, 4 * b + b] = 1.0
        sels[1, 4 * b + b] = 1.0
    csth = np.concatenate(
        [labh, glmh, np.broadcast_to(karr_h[None, 0], (B, 4)),
         i4h.reshape(B, BPC),
         np.broadcast_to(selu[None], (NCORES, BPC, 4 * BPC)).reshape(B, -1),
         np.broadcast_to(sels[None], (NCORES, BPC, 4 * BPC)).reshape(B, -1)],
        axis=1,
    ).astype(np.float32)  # (B, 552)
    in_maps = []
    for c in range(NCORES):
        s = slice(BPC * c, BPC * (c + 1))
        in_maps.append({
            "xt": np.ascontiguousarray(tf_[s]),
            "xs": np.ascontiguousarray(sf[s]),
            "pq": pqh,
            "cst": np.ascontiguousarray(csth[s]),
        })
    return in_maps


def run(inputs, trace=False, **kwargs):
    if "nc" not in _CACHE:
        _CACHE["nc"] = build()
    nc = _CACHE["nc"]
    res = run_bass_kernel_spmd(
        nc, make_in_maps(inputs), core_ids=list(range(NCORES)), trace=trace, **kwargs
    )
    outs = [res.results[c]["out"].reshape(BPC, 1, HS, WS) for c in range(NCORES)]
    return np.concatenate(outs, axis=0), res


def kernel(**inputs) -> np.ndarray:
    out, _ = run(inputs)
    return out


# revision 25
# speedup vs baseline: 2.4590x; 1.1954x over previous
"""Bass/Trainium2 kernel for nn_DiscriminativeCorrelationFilter.

Math
----
Reference computes, per batch b:
  sp = BN(W @ xs_b), tp = BN(W @ xt_b)        (1x1 conv 768->768 + eval-mode BN)
  label from mask centroid (Gaussian)
  f_0 = f_init;  5 iterations:
      r = f_t . tp  (per pixel);  cond = (r*label < 1)
      grad_b = mean(cond * (-label*mask))     (a SCALAR per batch)
      f_{t+1} = (1-LR*LAM) f_t - LR*grad_b*ones
  out_b = f_5 . sp

Because BN(W@x) = inv_std .* (W@x) + cvec (affine per channel) and f_t
stays in span{f_init, ones} (the gradient is a per-batch scalar):
  f_t = a_t * f_init + c_t * ones,  a_t = rho^t  (compile-time)
every channel contraction collapses onto two fixed vectors
    p = W^T (f_init .* inv_std),  q = W^T inv_std          (768 each)
with scalars k1 = f_init.cvec, k2 = sum(cvec):
    f_t . BN(W@x) = a_t (p^T x + k1) + c_t (q^T x + k2)
Device work per batch (features streamed as fp16, ~8 MB/core total):
  target:  psT = [p;q]^T @ xt  (M=2 matmuls), transposed to
           batch-on-partition layout via tiny selection matmuls
  recurrence on ctil_t = c_t/a_t, incremental form (2 DVE ops/iter):
    resp_t = resp_{t-1} + delta_t * (s*lab)
    delta_{t+1} = sum((resp_t < rho^-t) * glm * rho^-(t+1))  [accum_out]
    ctil5 = sum(delta_t)
  search:  bank_h += [p;q]^T @ xs chunks as they stream in (M=2,
           4 chains per PSUM bank via tile_position col-groups) --
           independent of the recurrence, so PE overlaps the DMA
  combine: W3_b = [1; ctil5_b; a5 k1 + a5 k2 ctil5_b] built on-chip
           (diag trick through an identity matmul);
           out_(b,h) = W3_b^T @ [a5 P; a5 Q; ones]  (K=3 matmul)
All weight-derived vectors (p, q, k1, k2, label, glm) are cheap host
precomputes from the small replicated weights (a 768x768 matvec);
the output is exactly f5 . BN(W@xs) re-associated, so the 48 GFLOP of
768x768 projections never run: the kernel is DMA/PE-overlap bound.

Sharding: data-parallel over batch, 4 batches per core on 8 cores.
Engine-op SBUF operands stay at partition bases in {0,32,64,96} (HW
restriction); all other partition rearrangement is done with tiny
selection/identity matmuls on the PE.
"""

import time

import numpy as np
from contextlib import ExitStack

import concourse.bacc as bacc
import concourse.mybir as mybir
import concourse.tile as tile
from concourse.bass_utils import run_bass_kernel_spmd

# ---------------- problem constants (hardcoded; kernel.py must be standalone)
B = 32            # full batch
D = 768           # feature dim
HS = WS = 32      # search spatial
HT = WT = 16      # target spatial
NS = HS * WS      # 1024
NT = HT * WT      # 256
NCORES = 8
BPC = B // NCORES  # 4 batches per core
KC = D // 128      # 6 contraction chunks

LR = 0.1
LAM = 0.01
SIGMA = 2.0
NIT = 5
BN_EPS = 1e-5
RHO = 1.0 - LR * LAM          # 0.999
A5 = RHO ** NIT

F32 = mybir.dt.float32
F16 = mybir.dt.float16   # features stream as fp16 (2-byte, fine mantissa)

_CACHE = {}
XS_DT = F16


def build():
    """Build the per-core Bass program (shapes only; no input values baked)."""
    nc = bacc.Bacc()
    XT_DT = F16
    xt = nc.dram_tensor("xt", (BPC, D, NT), XT_DT, kind="ExternalInput")
    xs = nc.dram_tensor("xs", (BPC, D, NS), XS_DT, kind="ExternalInput")
    cst = nc.dram_tensor("cst", (BPC, 6 * NT + 40), F32, kind="ExternalInput")
    out = nc.dram_tensor("out", (BPC, NS), F32, kind="ExternalOutput")

    AL = mybir.AluOpType
    AF = mybir.ActivationFunctionType

    with tile.TileContext(nc) as tc, ExitStack() as ctx:
        const = ctx.enter_context(tc.tile_pool(name="const", bufs=1))
        feats = ctx.enter_context(tc.tile_pool(name="feats", bufs=1))
        work = ctx.enter_context(tc.tile_pool(name="work", bufs=1))
        psum = ctx.enter_context(tc.tile_pool(name="psum", bufs=8, space="PSUM"))

        # ---- small constant loads
        pqb = nc.dram_tensor("pqb", (D, 2), XS_DT, kind="ExternalInput")
        pqb_sb = const.tile([128, KC, 2], XS_DT, tag="pqb")
        nc.scalar.dma_start(pqb_sb[:, :, :], pqb.rearrange("(k p) c -> p k c", p=128))
        cst_sb = const.tile([BPC, 6 * NT + 40], F32, tag="cst")
        nc.scalar.dma_start(cst_sb[:, :], cst[:, :])
        lab_sb = cst_sb[:, 0:NT]
        glmt_sb = [cst_sb[:, (1 + t) * NT:(2 + t) * NT] for t in range(NIT)]
        karr_sb = cst_sb[:, 6 * NT:6 * NT + 4]
        i4_sb = cst_sb[:, 6 * NT + 4:6 * NT + 8]
        selu_sb = cst_sb[:, 6 * NT + 8:6 * NT + 24]
        sels_sb = cst_sb[:, 6 * NT + 24:6 * NT + 40]

        # ---- feature loads (target first: it gates the serial recurrence)
        xt_sb = []
        for k in range(KC):
            t = feats.tile([128, BPC, NT], XT_DT, tag=f"xt{k}", name=f"xt{k}")
            nc.sync.dma_start(
                t[:, :, :], xt[:, k * 128:(k + 1) * 128, :].rearrange("b p n -> p b n")
            )
            xt_sb.append(t)
        xs_sb = []
        for k in range(KC):
            t = feats.tile([128, BPC, NS], XS_DT, tag=f"xs{k}", name=f"xs{k}")
            nc.sync.dma_start(
                t[:, :, :], xs[:, k * 128:(k + 1) * 128, :].rearrange("b p n -> p b n")
            )
            xs_sb.append(t)

        # ---- target stage: psT[j] (2,512) = [p;q]^T @ xt for batches (2j, 2j+1)
        psT = [psum.tile([2, 512], F32, tag="ps", name=f"psT{j}") for j in range(2)]
        for j in range(2):
            for k in range(KC):
                nc.tensor.matmul(
                    psT[j][:, :],
                    pqb_sb[:, k, :],
                    xt_sb[k][:, 2 * j:2 * j + 2, :],
                    start=(k == 0),
                    stop=(k == KC - 1),
                )

        # ---- move rows to batch-on-partition layout via SBUF->SBUF DMA
        PQs = work.tile([2, 2 * 512], F32, tag="PQs")
        for j in range(2):
            nc.scalar.copy(PQs[:, j * 512:(j + 1) * 512], psT[j][:, :])
        # selection matmuls transpose the psT rows into batch-on-partition
        # PSUM tiles (no SBUF->SBUF DMA latency): psU[b, :] = PQs[0, b-block]
        psU = psum.tile([BPC, NT], F32, tag="ps", name="psU")
        psSv = psum.tile([BPC, NT], F32, tag="ps", name="psSv")
        for b in range(BPC):
            nc.tensor.matmul(
                psU[:, :], selu_sb[0:2, 4 * b:4 * b + 4],
                PQs[0:2, b * NT:(b + 1) * NT],
                start=(b == 0), stop=(b == BPC - 1),
            )
        for b in range(BPC):
            nc.tensor.matmul(
                psSv[:, :], sels_sb[0:2, 4 * b:4 * b + 4],
                PQs[0:2, b * NT:(b + 1) * NT],
                start=(b == 0), stop=(b == BPC - 1),
            )

        # Ulab = (psU + k1) * label ; Slab = (psSv + k2) * label
        Ulab = work.tile([BPC, NT], F32, tag="Ulab")
        Slab = work.tile([BPC, NT], F32, tag="Slab")
        nc.vector.scalar_tensor_tensor(
            Ulab[:, :], psU[:, :], karr_sb[:, 0:1], lab_sb, AL.add, AL.mult
        )
        nc.vector.scalar_tensor_tensor(
            Slab[:, :], psSv[:, :], karr_sb[:, 1:2], lab_sb, AL.add, AL.mult
        )

        # ---- 5-iteration recurrence: resp_t = resp_{t-1} + delta_t*Slab,
        # delta_t = sum(cond_{t-1} * glm * rho^-t) (glm pre-scaled on host)
        resp = work.tile([BPC, NT], F32, tag="resp")
        junk = work.tile([BPC, NT], F32, tag="junk")
        Gt = work.tile([BPC, NIT], F32, tag="Gt")
        nc.vector.scalar_tensor_tensor(
            junk[:, :], Ulab[:, :], 1.0, glmt_sb[0], AL.is_lt, AL.mult,
            accum_out=Gt[:, 0:1],
        )
        for t in range(1, NIT):
            nc.vector.scalar_tensor_tensor(
                resp[:, :], Slab[:, :], Gt[:, t - 1:t],
                Ulab[:, :] if t == 1 else resp[:, :], AL.mult, AL.add
            )
            nc.vector.scalar_tensor_tensor(
                junk[:, :], resp[:, :], float(RHO ** -t), glmt_sb[t],
                AL.is_lt, AL.mult, accum_out=Gt[:, t:t + 1],
            )
        ctil5 = work.tile([BPC, 1], F32, tag="ctil5")
        nc.vector.reduce_sum(ctil5[:, :], Gt[:, :], axis=mybir.AxisListType.X)

        # ---- search stage: [p;q]^T @ xs chunks, 4 chains per PSUM bank
        # (col-group packing: chain (b,h) lives at rows 32b..32b+1 of bank h)
        bank = [psum.tile([128, 512], F32, tag="ps", name=f"bank{h}")
                for h in range(2)]
        for k in range(KC):
            for b in range(BPC):
                for h in range(2):
                    nc.tensor.matmul(
                        bank[h][32 * b:32 * b + 2, :],
                        pqb_sb[:, k, :],
                        xs_sb[k][:, b, h * 512:(h + 1) * 512],
                        tile_position=(0, 32 * b),
                        start=(k == 0),
                        stop=(k == KC - 1),
                    )

        # ---- W3 = per-batch combine weights [1; ctil5; kb] via WL/I4 matmul
        WL = work.tile([BPC, 3], F32, tag="WL")
        nc.vector.memset(WL[:, 0:1], 1.0)
        nc.vector.tensor_scalar(
            out=WL[:, 1:2], in0=ctil5[:, :], scalar1=1.0, scalar2=None, op0=AL.mult,
        )
        nc.vector.tensor_scalar(
            out=WL[:, 2:3], in0=ctil5[:, :], scalar1=karr_sb[:, 3:4],
            scalar2=karr_sb[:, 2:3], op0=AL.mult, op1=AL.add,
        )
        W3ps = psum.tile([3, BPC], F32, tag="ps", name="W3ps")
        nc.tensor.matmul(W3ps[:, :], WL[:, :], i4_sb, start=True, stop=True)
        W3_sb = work.tile([3, BPC], XS_DT, tag="W3_sb")
        nc.vector.tensor_copy(W3_sb[:, :], W3ps[:, :])

        # ---- combine: out_(b,h) = W3_b^T @ [a5*P; a5*Q; ones] then copy out
        PQc = [work.tile([3, 512], XS_DT, tag=f"PQc{i}", name=f"PQc{i}")
               for i in range(2)]
        for i in range(2):
            nc.vector.memset(PQc[i][:, :], 1.0)
        out_row = work.tile([1, BPC, 2, 512], F32, tag="out_row")
        psF = [psum.tile([1, 512], F32, tag="ps", name=f"psF{i}")
               for i in range(2 * BPC)]
        for b in range(BPC):
            for h in range(2):
                j = b * 2 + h
                src_ap = bank[h][32 * b:32 * b + 2, :]
                dst_ap = PQc[j % 2][0:2, :]
                if j % 2 == 0:
                    nc.scalar.activation(dst_ap, src_ap, AF.Copy,
                                         scale=float(A5))
                else:
                    nc.vector.tensor_scalar(
                        out=dst_ap, in0=src_ap, scalar1=float(A5),
                        scalar2=None, op0=AL.mult,
                    )
                nc.tensor.matmul(psF[j][:, :], W3_sb[:, b:b + 1],
                                 PQc[j % 2][:, :], start=True, stop=True)
                if j % 2 == 0:
                    nc.vector.tensor_copy(out_row[0:1, b, h, :], psF[j][0:1, :])
                else:
                    nc.scalar.copy(out_row[0:1, b, h, :], psF[j][0:1, :])
        nc.sync.dma_start(out[0:2, :], out_row[0:1, 0:2, :, :])
        nc.sync.dma_start(out[2:4, :], out_row[0:1, 2:4, :, :])

    nc.finalize()
    return nc


def _host_prep(inputs):
    """Host-side precomputation of p, q, k1, k2, label, glm from small weights."""
    mask = np.asarray(inputs["target_mask"], np.float32).reshape(B, NT)
    W = np.asarray(inputs["conv_w"], np.float64)
    cb = np.asarray(inputs["conv_b"], np.float64)
    gamma = np.asarray(inputs["bn_gamma"], np.float64)
    beta = np.asarray(inputs["bn_beta"], np.float64)
    mean = np.asarray(inputs["bn_mean"], np.float64)
    var = np.asarray(inputs["bn_var"], np.float64)
    f0 = np.asarray(inputs["filter_init"], np.float64).reshape(D)

    inv_std = gamma / np.sqrt(var + BN_EPS)
    cvec = (cb - mean) * inv_std + beta
    p = W.T @ (f0 * inv_std)
    q = W.T @ inv_std
    k1 = float(f0 @ cvec)
    k2 = float(cvec.sum())
    pqh = np.stack([p, q], axis=1).astype(np.float32)          # (768, 2)
    karr_row = np.array([k1, k2, A5 * k1, A5 * k2], np.float64).astype(np.float32)
    karr_h = np.broadcast_to(karr_row, (BPC, 4)).copy()

    # Gaussian label from mask centroid (float32 to mirror the fp32 reference)
    yy, xx = np.meshgrid(
        np.arange(HT, dtype=np.float32), np.arange(WT, dtype=np.float32), indexing="ij"
    )
    yf, xf = yy.reshape(-1), xx.reshape(-1)
    msum = np.maximum(mask.sum(1), np.float32(1.0))
    cy = (mask * yf).sum(1) / msum
    cx = (mask * xf).sum(1) / msum
    d2 = (xf[None, :] - cx[:, None]) ** 2 + (yf[None, :] - cy[:, None]) ** 2
    labh = np.exp(-d2 / np.float32(2.0 * SIGMA * SIGMA)).astype(np.float32)
    glmh = (np.float32(LR / NT) * labh * mask).astype(np.float32)
    glmth = [(glmh * np.float32(RHO ** -(t + 1))).astype(np.float32)
             for t in range(NIT)]
    return pqh, karr_h, labh, glmth


def make_in_maps(inputs):
    sf = np.asarray(inputs["search_features"], np.float32).reshape(B, D, NS)
    sf = sf.astype(np.float16)
    sf = np.ascontiguousarray(sf)
    tf_ = np.asarray(inputs["target_features"], np.float32).reshape(B, D, NT)
    tf_ = tf_.astype(np.float16)
    tf_ = np.ascontiguousarray(tf_)
    pqh, karr_h, labh, glmth = _host_prep(inputs)
    i4h = np.broadcast_to(np.eye(BPC, dtype=np.float32)[None], (NCORES, BPC, BPC))
    # selection matrices, rows 0-1 meaningful: selU[0, 4b+m] = (m == b)
    selu = np.zeros((BPC, 4 * BPC), np.float32)
    sels = np.zeros((BPC, 4 * BPC), np.float32)
    for b in range(BPC):
        selu[0, 4 * b + b] = 1.0
        sels[1, 4 * b + b] = 1.0
    csth = np.concatenate(
        [labh] + glmth +
        [np.broadcast_to(karr_h[None, 0], (B, 4)),
         i4h.reshape(B, BPC),
         np.broadcast_to(selu[None], (NCORES, BPC, 4 * BPC)).reshape(B, -1),
         np.broadcast_to(sels[None], (NCORES, BPC, 4 * BPC)).reshape(B, -1)],
        axis=1,
    ).astype(np.float32)  # (B, 1576)
    in_maps = []
    for c in range(NCORES):
        s = slice(BPC * c, BPC * (c + 1))
        in_maps.append({
            "xt": np.ascontiguousarray(tf_[s]),
            "xs": np.ascontiguousarray(sf[s]),
            "pqb": pqh.astype(np.float16),
            "cst": np.ascontiguousarray(csth[s]),
        })
    return in_maps


def run(inputs, trace=False, **kwargs):
    if "nc" not in _CACHE:
        _CACHE["nc"] = build()
    nc = _CACHE["nc"]
    in_maps = make_in_maps(inputs)
    last_err = None
    for _attempt in range(3):
        try:
            res = run_bass_kernel_spmd(
                nc, in_maps, core_ids=list(range(NCORES)), trace=trace, **kwargs
            )
            break
        except Exception as e:  # transient NRT device faults recover on retry
            last_err = e
            time.sleep(2.0)
    else:
        raise last_err
    outs = [res.results[c]["out"].reshape(BPC, 1, HS, WS) for c in range(NCORES)]
    return np.concatenate(outs, axis=0), res


def kernel(**inputs) -> np.ndarray:
    out, _ = run(inputs)
    return out
